# revision 1
# baseline (speedup 1.0000x reference)
"""Trainium2 Bass kernel for GNN message passing.

Computes, for full inputs:
    edge_h = lrelu(lrelu(edge @ We1 + be1) @ We2 + be2)        # [E, 128]
    out    = segment_sum((node @ Wn)[idx_j] * edge_h, seg_i)   # [N, 128]

Strategy (8 NeuronCores, SPMD single program):
  - Shard edges by DESTINATION range (N/8 = 6250 nodes per core). Each core
    produces its own output slice -> no collectives; host concatenates.
  - Phase 1 (replicated on every core): msg = node @ Wn written to two
    internal DRAM tables (halves, so gather indices fit int16), row-major
    fp32 (512B rows -> full-rate gather).
  - Phase 2: edges host-sorted by destination into 128-node windows; within
    a window, edges are grouped by which msg half their source lives in.
    Source rows are fetched with dma_gather (InstDMAGatherAnt) in long
    "runs" of up to 16 tiles sharing one table. Per 128-edge tile: edge MLP
    via PE matmuls (edge features pre-transposed on host), LeakyReLU (Prelu)
    on the scalar engine, one-hot(slot) built on DVE via is_equal vs iota,
    and scatter-add = one-hot matmul accumulated in a PSUM window
    [128 slots x 128 feat]; window flushed to DRAM when done.

The tile->window/table schedule is data-dependent and baked into the
program at build time (all cores share it; per-core data is padded to the
common schedule). Programs are cached per schedule within the process.
"""

import os
import sys
import types

import numpy as np

import concourse.bass as bass
import concourse.tile as tile
from concourse.tile import add_dep_helper
from concourse import bacc, mybir
from concourse.bass_utils import run_bass_kernel_spmd

# ---- problem constants (hardcoded per spec) ----
N_NODES = 50000
D_NODE = 128
D_EDGE = 32
D_HID = 128
N_CORES = 8
NPC = N_NODES // N_CORES          # nodes per core = 6250
P = 128                           # partitions
W_PER_CORE = (NPC + P - 1) // P   # 49 windows per core
NEG_SLOPE = 0.01
PAD_SLOT = 300.0                  # slot value that never matches iota 0..127

# node rows padded to a multiple of 1024 so the two msg halves split evenly
N_PAD = ((N_NODES + 1023) // 1024) * 1024    # 50176
N_SLABS = N_PAD // 512                       # 98
HALF = N_PAD // 2                            # 25088 rows per msg table
HALF_SLABS = N_SLABS // 2                    # 49

RUN_MAX = 8                       # max tiles per dma_gather (1024 idxs; >1024 crashes)

F32 = mybir.dt.float32
F16 = mybir.dt.float16
I16 = mybir.dt.int16
# data dtype for matmul operands / msg tables (fp16 default; K_DT=f32 to revert)
DT = F32 if os.environ.get("K_DT") == "f32" else F16
DT_NP = np.float32 if os.environ.get("K_DT") == "f32" else np.float16

TRACE = False
LAST_RESULT = None

_PROGRAM_CACHE = {}


def _ensure_ntff_hook():
    """Provide antenv.axon_hooks if this image's antenv lacks it, and
    register the ctypes NTFF profiling hook so trace=True works."""
    try:
        from antenv.axon_hooks import get_axon_ntff_profile_hook  # noqa: F401
        return
    except ImportError:
        pass
    mod = types.ModuleType("antenv.axon_hooks")
    _hook = [None]
    mod.set_axon_ntff_profile_hook = lambda h: _hook.__setitem__(0, h)
    mod.get_axon_ntff_profile_hook = lambda: _hook[0]
    sys.modules["antenv.axon_hooks"] = mod
    import antenv

    antenv.axon_hooks = mod
    try:
        from trn_agent_boot.trn_boot import _ntff_profile_via_ctypes

        mod.set_axon_ntff_profile_hook(
            _ntff_profile_via_ctypes("/opt/axon/libaxon_pjrt.so")
        )
    except Exception:
        pass


# --------------------------------------------------------------------------
# host-side schedule + packing
# --------------------------------------------------------------------------

def cdiv(a, b):
    return (a + b - 1) // b


class Schedule:
    """Common (all-core) static schedule baked into the program."""

    def __init__(self, seg_i, idx_j):
        core = seg_i // NPC
        local = seg_i - core * NPC
        win = local // P
        half = (idx_j >= HALF).astype(np.int64)

        cnt = np.zeros((2, N_CORES, W_PER_CORE), dtype=np.int64)
        for c in range(N_CORES):
            m = core == c
            for h in (0, 1):
                cnt[h, c] = np.bincount(
                    win[m & (half == h)], minlength=W_PER_CORE
                )
        a_tiles = cdiv(cnt[0].max(axis=0), P)
        b_tiles = cdiv(cnt[1].max(axis=0), P)
        empty = (a_tiles + b_tiles) == 0
        b_tiles[empty] = 1

        # tile sequence; orientation alternates so same-table runs merge
        win_of, table_of, first_of, last_of = [], [], [], []
        # per (w, table): [start_tile, n_tiles] for packing
        self.block_start = np.zeros((W_PER_CORE, 2), dtype=np.int64)
        self.block_tiles = np.zeros((W_PER_CORE, 2), dtype=np.int64)
        for w in range(W_PER_CORE):
            order = (0, 1) if w % 2 == 0 else (1, 0)
            nts = {0: int(a_tiles[w]), 1: int(b_tiles[w])}
            for tab in order:
                self.block_start[w, tab] = len(win_of)
                self.block_tiles[w, tab] = nts[tab]
                for _ in range(nts[tab]):
                    win_of.append(w)
                    table_of.append(tab)
            s, e = None, None
        # first/last flags per window from the sequence
        win_of = np.array(win_of, dtype=np.int64)
        T = len(win_of)
        first_of = np.zeros(T, dtype=bool)
        last_of = np.zeros(T, dtype=bool)
        for w in range(W_PER_CORE):
            ids = np.flatnonzero(win_of == w)
            first_of[ids[0]] = True
            last_of[ids[-1]] = True

        self.T = T
        self.win_of = win_of
        self.table_of = np.array(table_of, dtype=np.int64)
        self.first_of = first_of
        self.last_of = last_of
        self.core, self.local, self.win, self.half = core, local, win, half

        # gather runs: maximal same-table spans, capped at RUN_MAX
        runs = []
        t = 0
        while t < T:
            tab = self.table_of[t]
            e = t
            while e < T and self.table_of[e] == tab and e - t < RUN_MAX:
                e += 1
            runs.append((int(tab), t, e - t))
            t = e
        self.runs = runs

        # chunks: <=4-tile pieces within runs
        chunks = []
        for ri, (tab, t0, L) in enumerate(runs):
            t = t0
            while t < t0 + L:
                nt = min(4, t0 + L - t)
                chunks.append((ri, t, nt))
                t += nt
        self.chunks = chunks

    def key(self):
        return (
            tuple(self.win_of.tolist()),
            tuple(self.table_of.tolist()),
        )


def _pack_core(c, S, edge, idx_j):
    """Per-core padded arrays following the common schedule."""
    T = S.T
    perm = np.full(T * P, -1, dtype=np.int64)
    for w in range(W_PER_CORE):
        for tab in (0, 1):
            sel = np.flatnonzero(
                (S.core == c) & (S.win == w) & (S.half == tab)
            )
            n = len(sel)
            s0 = S.block_start[w, tab] * P
            cap = S.block_tiles[w, tab] * P
            assert n <= cap, f"schedule overflow c={c} w={w} tab={tab}"
            perm[s0:s0 + n] = sel

    valid = perm >= 0
    pidx = np.where(valid, perm, 0)

    # idx16 [128, T*8]: per tile-order flattening i=t*128+p -> [i%16, i//16]
    loc = (idx_j[pidx] - S.table_of[np.arange(T * P) // P] * HALF).astype(
        np.int64
    )
    loc[~valid] = 0
    assert (loc >= 0).all() and (loc < HALF).all()
    idx16 = loc.astype(np.int16).reshape(T * 8, 16).T  # [16, T*8]
    idx16 = np.tile(idx16, (8, 1)).copy()              # [128, T*8]

    # chunk-packed edge features and slots
    n_ch = len(S.chunks)
    edgeT = np.zeros((n_ch, D_EDGE, 512), dtype=DT_NP)
    slotp = np.full((n_ch, P, 4), PAD_SLOT, dtype=DT_NP)
    slots_all = (S.local[pidx] % P).astype(DT_NP)
    slots_all[~valid] = PAD_SLOT
    ev = np.zeros((T * P, D_EDGE), dtype=DT_NP)
    ev[valid] = edge[pidx[valid]]
    for k, (ri, t0, nt) in enumerate(S.chunks):
        edgeT[k, :, :nt * P] = ev[t0 * P:(t0 + nt) * P].T
        for t in range(nt):
            slotp[k, :, t] = slots_all[(t0 + t) * P:(t0 + t + 1) * P]
    return edgeT, slotp, idx16


# --------------------------------------------------------------------------
# device program
# --------------------------------------------------------------------------

def _build_program(S):
    T = S.T
    n_ch = len(S.chunks)
    dbg_skip_p1 = os.environ.get("K_SKIP_P1") == "1"
    dbg_skip_gather = os.environ.get("K_SKIP_GATHER") == "1"
    dbg_max_chunks = int(os.environ.get("K_MAX_CHUNKS", "999999"))
    dbg_stages = int(os.environ.get("K_STAGES", "99"))

    nc = bacc.Bacc(
        "TRN2", target_bir_lowering=False, debug=False, num_devices=N_CORES
    )

    # ---- I/O ----
    nodeT_h = nc.dram_tensor("nodeT", [P, N_PAD], DT, kind="ExternalInput").ap()
    Wn_h = nc.dram_tensor("Wn", [D_NODE, D_HID], DT, kind="ExternalInput").ap()
    We1_h = nc.dram_tensor("We1p", [D_EDGE, D_HID], DT, kind="ExternalInput").ap()
    We2_h = nc.dram_tensor("We2", [D_HID, D_HID], DT, kind="ExternalInput").ap()
    be1_h = nc.dram_tensor("be1c", [P, 1], F32, kind="ExternalInput").ap()
    be2_h = nc.dram_tensor("be2bc", [P, 512], F32, kind="ExternalInput").ap()
    iota_h = nc.dram_tensor("iota", [P, P], DT, kind="ExternalInput").ap()
    edgeT_h = nc.dram_tensor(
        "edgeT", [n_ch, D_EDGE, 512], DT, kind="ExternalInput"
    ).ap()
    slot_h = nc.dram_tensor(
        "slotp", [n_ch, P, 4], DT, kind="ExternalInput"
    ).ap()
    idx16_h = nc.dram_tensor(
        "idx16", [P, T * 8], I16, kind="ExternalInput"
    ).ap()
    out_h = nc.dram_tensor(
        "out", [W_PER_CORE * P, D_HID], F32, kind="ExternalOutput"
    ).ap()

    msg_h = [
        nc.dram_tensor("msgA", [HALF, D_HID], DT).ap(),
        nc.dram_tensor("msgB", [HALF, D_HID], DT).ap(),
    ]

    LR = mybir.ActivationFunctionType.Prelu

    with tile.TileContext(nc) as tc:
        with tc.tile_pool(name="consts", bufs=1) as cpool:
            Wn_sb = cpool.tile([D_NODE, D_HID], DT)
            nc.sync.dma_start(Wn_sb[:], Wn_h[:])
            We1_sb = cpool.tile([D_EDGE, D_HID], DT)
            nc.sync.dma_start(We1_sb[:], We1_h[:])
            We2_sb = cpool.tile([D_HID, D_HID], DT)
            nc.sync.dma_start(We2_sb[:], We2_h[:])
            be1_sb = cpool.tile([P, 1], F32)
            nc.sync.dma_start(be1_sb[:], be1_h[:])
            be2_sb = cpool.tile([P, 512], F32)
            nc.sync.dma_start(be2_sb[:], be2_h[:])
            iota_sb = cpool.tile([P, P], DT)
            nc.sync.dma_start(iota_sb[:], iota_h[:])

            # ---- phase 1: msg = node @ Wn (slabs of 512 rows) ----
            p1_stores = []
            with (
                tc.tile_pool(name="p1_in", bufs=3) as p1in,
                tc.tile_pool(name="p1_stage", bufs=3) as p1st,
                tc.tile_pool(name="p1_psum", bufs=2, space="PSUM") as p1ps,
            ):
                for g in range(0 if dbg_skip_p1 else N_SLABS):
                    nt_sb = p1in.tile([P, 512], DT, tag="nodeT")
                    nc.sync.dma_start(
                        nt_sb[:], nodeT_h[:, g * 512:(g + 1) * 512]
                    )
                    ps = p1ps.tile([P, 512], F32, tag="p1ps")
                    for t in range(4):
                        nc.tensor.matmul(
                            ps[:, t * P:(t + 1) * P],
                            lhsT=nt_sb[:, t * P:(t + 1) * P],
                            rhs=Wn_sb[:],
                            start=True,
                            stop=True,
                        )
                    stage = p1st.tile([P, 512], DT, tag="p1stage")
                    if g % 2 == 0:
                        nc.vector.tensor_copy(stage[:], ps[:])
                    else:
                        nc.scalar.activation(
                            stage[:], ps[:],
                            mybir.ActivationFunctionType.Copy,
                        )
                    dst_tab = msg_h[0] if g < HALF_SLABS else msg_h[1]
                    r0 = (g % HALF_SLABS) * 512
                    dst = dst_tab[r0:r0 + 512, :].rearrange(
                        "(t p) f -> p t f", p=P
                    )
                    srcap = stage[:].rearrange("p (t f) -> p t f", t=4)
                    st_inst = nc.sync.dma_start(dst, srcap)
                    p1_stores.append(st_inst.ins)

            # ---- phase 2 ----
            with (
                tc.tile_pool(name="p2_in", bufs=3) as p2in,
                tc.tile_pool(name="p2_g", bufs=2) as p2g,
                tc.tile_pool(name="p2_mid", bufs=3) as p2mid,
                tc.tile_pool(name="h1_psum", bufs=2, space="PSUM") as h1ps,
                tc.tile_pool(name="h2_psum", bufs=2, space="PSUM") as h2ps,
                tc.tile_pool(name="out_psum", bufs=2, space="PSUM") as outps,
                tc.tile_pool(name="out_stage", bufs=3) as outst,
            ):
                cur_out = {}
                G_of_run = {}

                chunks_by_run = {}
                for k, (ri, t0, nt) in enumerate(S.chunks):
                    chunks_by_run.setdefault(ri, []).append((k, t0, nt))

                for ri, (tab, rt0, L) in enumerate(S.runs):
                    ixr = p2in.tile(
                        [P, RUN_MAX * 8], I16, tag="ix", name=f"ix_r{ri}"
                    )
                    nc.sync.dma_start(
                        ixr[:, :L * 8],
                        idx16_h[:, rt0 * 8:(rt0 + L) * 8],
                    )
                    G = p2g.tile(
                        [P, RUN_MAX * P], DT, tag="G", name=f"G_r{ri}"
                    )
                    if dbg_skip_gather:
                        nc.gpsimd.memset(G[:, :L * P], 0.5)
                    else:
                        g_inst = nc.gpsimd.dma_gather(
                            G[:, :L * P].rearrange("p (g f) -> p g f", f=P),
                            msg_h[tab][:],
                            ixr[:, :L * 8],
                            num_idxs=L * P,
                            num_idxs_reg=L * P,
                            elem_size=P,
                            elem_step=P,
                        )
                        if ri == 0:
                            for st in p1_stores:
                                add_dep_helper(
                                    g_inst.ins, st, sync=True,
                                    reason="gather after msg stores",
                                )

                    for (k, t0, nt) in chunks_by_run[ri]:
                        if k >= dbg_max_chunks:
                            continue
                        ncols = nt * P
                        goff = (t0 - rt0) * P

                        et_sb = p2in.tile([D_EDGE, 512], DT, tag="edgeT")
                        nc.sync.dma_start(
                            et_sb[:, :ncols], edgeT_h[k, :, :ncols]
                        )
                        sl_sb = p2in.tile([P, 4], DT, tag="slot")
                        nc.sync.dma_start(sl_sb[:, :nt], slot_h[k, :, :nt])

                        # h1 = lrelu(edge @ We1 + be1), feature-major [h x e]
                        ps1 = h1ps.tile([P, 512], F32, tag="h1ps")
                        nc.tensor.matmul(
                            ps1[:, :ncols],
                            lhsT=We1_sb[:],
                            rhs=et_sb[:, :ncols],
                            start=True,
                            stop=True,
                        )
                        h1f = p2mid.tile([P, 512], DT, tag="h1f")
                        if dbg_stages >= 2:
                            nc.scalar.activation(
                                h1f[:, :ncols], ps1[:, :ncols], LR,
                                bias=be1_sb[:], scale=1.0, alpha=NEG_SLOPE,
                            )
                        else:
                            nc.vector.tensor_copy(h1f[:, :ncols], ps1[:, :ncols])
                        if dbg_stages < 3:
                            continue

                        # h2 = h1.T @ We2 + be2, edge-major [e x h]
                        ps2 = h2ps.tile([P, 512], F32, tag="h2ps")
                        for t in range(nt):
                            nc.tensor.matmul(
                                ps2[:, t * P:(t + 1) * P],
                                lhsT=h1f[:, t * P:(t + 1) * P],
                                rhs=We2_sb[:],
                                start=True,
                                stop=True,
                            )
                        nc.vector.tensor_tensor(
                            ps2[:, :ncols], in0=ps2[:, :ncols],
                            in1=be2_sb[:, :ncols], op=mybir.AluOpType.add,
                        )
                        eh = p2mid.tile([P, 512], DT, tag="eh")
                        if dbg_stages >= 4:
                            nc.scalar.activation(
                                eh[:, :ncols], ps2[:, :ncols], LR,
                                scale=1.0, alpha=NEG_SLOPE,
                            )
                        else:
                            nc.vector.tensor_copy(eh[:, :ncols], ps2[:, :ncols])
                        if dbg_stages < 5:
                            continue

                        # onehot[e, s] = (slot[e] == s)
                        oh = p2mid.tile([P, 512], DT, tag="oh")
                        if dbg_stages >= 5:
                            for t in range(nt):
                                nc.vector.tensor_tensor(
                                    oh[:, t * P:(t + 1) * P],
                                    in0=sl_sb[:, t:t + 1].to_broadcast([P, P]),
                                    in1=iota_sb[:],
                                    op=mybir.AluOpType.is_equal,
                                )
                        else:
                            nc.gpsimd.memset(oh[:, :ncols], 0.0)
                        if dbg_stages < 6:
                            continue

                        # product = gathered msg * edge_h
                        pr = p2mid.tile([P, 512], DT, tag="pr")
                        nc.vector.tensor_tensor(
                            pr[:, :ncols],
                            in0=G[:, goff:goff + ncols],
                            in1=eh[:, :ncols],
                            op=mybir.AluOpType.mult,
                        )

                        if dbg_stages < 7:
                            continue
                        # scatter: out_w[s, f] += onehot[:, t].T @ product[:, t]
                        for t in range(nt):
                            i = t0 + t
                            w = int(S.win_of[i])
                            if S.first_of[i]:
                                cur_out[w] = outps.tile(
                                    [P, P], F32, tag="outp", name=f"outp_w{w}"
                                )
                            nc.tensor.matmul(
                                cur_out[w][:],
                                lhsT=oh[:, t * P:(t + 1) * P],
                                rhs=pr[:, t * P:(t + 1) * P],
                                start=bool(S.first_of[i]),
                                stop=bool(S.last_of[i]),
                            )
                            if S.last_of[i]:
                                st = outst.tile(
                                    [P, P], F32, tag="outstage",
                                    name=f"outst_w{w}"
                                )
                                nc.vector.tensor_copy(st[:], cur_out[w][:])
                                nc.sync.dma_start(
                                    out_h[w * P:(w + 1) * P, :], st[:]
                                )
                                del cur_out[w]

    nc.compile()
    return nc


# --------------------------------------------------------------------------
# entry point
# --------------------------------------------------------------------------

def kernel(node, edge, Wn, We1, be1, We2, be2, seg_i, idx_j):
    global LAST_RESULT
    node = np.asarray(node, dtype=np.float32)
    edge = np.asarray(edge, dtype=np.float32)
    Wn = np.asarray(Wn, dtype=np.float32)
    We1 = np.asarray(We1, dtype=np.float32)
    be1 = np.asarray(be1, dtype=np.float32)
    We2 = np.asarray(We2, dtype=np.float32)
    be2 = np.asarray(be2, dtype=np.float32)
    seg_i = np.asarray(seg_i, dtype=np.int32)
    idx_j = np.asarray(idx_j, dtype=np.int32)

    S = Schedule(seg_i.astype(np.int64), idx_j.astype(np.int64))
    key = S.key()
    if key not in _PROGRAM_CACHE:
        _PROGRAM_CACHE[key] = _build_program(S)
    nc = _PROGRAM_CACHE[key]

    nodeT = np.zeros((P, N_PAD), dtype=DT_NP)
    nodeT[:, :N_NODES] = node.T
    iota = np.broadcast_to(np.arange(P, dtype=DT_NP), (P, P)).copy()
    common = {
        "nodeT": nodeT,
        "Wn": Wn.astype(DT_NP),
        "We1p": We1.astype(DT_NP),
        "We2": We2.astype(DT_NP),
        "be1c": be1.reshape(P, 1).copy(),
        "be2bc": np.broadcast_to(
            np.tile(be2, 4), (P, 512)
        ).astype(np.float32).copy(),
        "iota": iota,
    }
    in_maps = []
    for c in range(N_CORES):
        edgeT, slotp, idx16 = _pack_core(c, S, edge, idx_j)
        m = dict(common)
        m["edgeT"] = edgeT
        m["slotp"] = slotp
        m["idx16"] = idx16
        in_maps.append(m)

    if TRACE:
        _ensure_ntff_hook()
    res = run_bass_kernel_spmd(
        nc, in_maps, list(range(N_CORES)), trace=TRACE
    )
    LAST_RESULT = res
    out = np.concatenate(
        [res.results[c]["out"][:NPC] for c in range(N_CORES)], axis=0
    )
    return out.astype(np.float32)



# revision 2
# speedup vs baseline: 1.3881x; 1.3881x over previous
"""Trainium2 Bass kernel for GNN message passing.

Computes, for full inputs:
    edge_h = lrelu(lrelu(edge @ We1 + be1) @ We2 + be2)        # [E, 128]
    out    = segment_sum((node @ Wn)[idx_j] * edge_h, seg_i)   # [N, 128]

Strategy (8 NeuronCores, SPMD single program):
  - Shard edges by DESTINATION range (N/8 = 6250 nodes per core). Each core
    produces its own output slice -> no collectives; host concatenates.
  - Phase 1 (replicated on every core): msg = node @ Wn written to two
    internal DRAM tables (halves, so gather indices fit int16), row-major
    fp16 (256B rows -> full-rate gather). msgA slabs are written first so
    pass-A gathers can start while msgB is still being computed.
  - Phase 2, two passes: pass A processes every window's msgA-sourced edges
    (gathers read only msgA), pass B the msgB-sourced ones. Per 128-edge
    tile: edge MLP via PE matmuls, LeakyReLU (Prelu) on the scalar engine,
    one-hot(slot) on DVE, scatter-add = one-hot matmul accumulated in a
    PSUM window [128 slots x 128 feat]. Pass-A windows flush to SBUF
    accumulators; pass-B flush adds the accumulator and stores to DRAM.
  - Gathers (InstDMAGatherAnt) are issued round-robin on 4 SWDGE queues
    (independent Q7 core pairs) with 6 G buffers, so up to 4 descriptor
    generations overlap: ~3-4 ns/idx instead of ~9 ns/idx single-queue.

The tile->window/table schedule is data-dependent and baked into the
program at build time (all cores share it; per-core data is padded to the
common schedule). Programs are cached per schedule within the process.
"""

import os
import sys
import types

import numpy as np

import concourse.bass as bass
import concourse.tile as tile
from concourse.tile import add_dep_helper
from concourse import bacc, mybir
from concourse.bass_utils import run_bass_kernel_spmd

# ---- problem constants (hardcoded per spec) ----
N_NODES = 50000
D_NODE = 128
D_EDGE = 32
D_HID = 128
N_CORES = 8
NPC = N_NODES // N_CORES          # nodes per core = 6250
P = 128                           # partitions
W_PER_CORE = (NPC + P - 1) // P   # 49 windows per core
NEG_SLOPE = 0.01
PAD_SLOT = 300.0                  # slot value that never matches iota 0..127

# node rows padded to a multiple of 1024 so the two msg halves split evenly
N_PAD = ((N_NODES + 1023) // 1024) * 1024    # 50176
N_SLABS = N_PAD // 512                       # 98
HALF = N_PAD // 2                            # 25088 rows per msg table
HALF_SLABS = N_SLABS // 2                    # 49

RUN_MAX = 8          # max tiles per dma_gather (1024 idxs; ring cap, >1920 wedges)
N_QUEUES = 4         # SWDGE queues (Q7 core pairs) for parallel desc-gen

F32 = mybir.dt.float32
F16 = mybir.dt.float16
I16 = mybir.dt.int16
# data dtype for matmul operands / msg tables (fp16 default; K_DT=f32 to revert)
DT = F32 if os.environ.get("K_DT") == "f32" else F16
DT_NP = np.float32 if os.environ.get("K_DT") == "f32" else np.float16

TRACE = False
LAST_RESULT = None

_PROGRAM_CACHE = {}


def _ensure_ntff_hook():
    """Provide antenv.axon_hooks if this image's antenv lacks it, and
    register the ctypes NTFF profiling hook so trace=True works."""
    try:
        from antenv.axon_hooks import get_axon_ntff_profile_hook  # noqa: F401
        return
    except ImportError:
        pass
    mod = types.ModuleType("antenv.axon_hooks")
    _hook = [None]
    mod.set_axon_ntff_profile_hook = lambda h: _hook.__setitem__(0, h)
    mod.get_axon_ntff_profile_hook = lambda: _hook[0]
    sys.modules["antenv.axon_hooks"] = mod
    import antenv

    antenv.axon_hooks = mod
    try:
        from trn_agent_boot.trn_boot import _ntff_profile_via_ctypes

        mod.set_axon_ntff_profile_hook(
            _ntff_profile_via_ctypes("/opt/axon/libaxon_pjrt.so")
        )
    except Exception:
        pass


# --------------------------------------------------------------------------
# host-side schedule + packing
# --------------------------------------------------------------------------

def cdiv(a, b):
    return (a + b - 1) // b


class Schedule:
    """Common (all-core) static schedule baked into the program.

    Tile sequence = pass A (all windows, table 0) then pass B (table 1).
    """

    def __init__(self, seg_i, idx_j):
        core = seg_i // NPC
        local = seg_i - core * NPC
        win = local // P
        half = (idx_j >= HALF).astype(np.int64)

        cnt = np.zeros((2, N_CORES, W_PER_CORE), dtype=np.int64)
        for c in range(N_CORES):
            m = core == c
            for h in (0, 1):
                cnt[h, c] = np.bincount(
                    win[m & (half == h)], minlength=W_PER_CORE
                )
        a_tiles = np.maximum(cdiv(cnt[0].max(axis=0), P), 1)
        b_tiles = np.maximum(cdiv(cnt[1].max(axis=0), P), 1)

        # tile sequence: pass A (tab 0) then pass B (tab 1)
        win_of, table_of = [], []
        self.block_start = np.zeros((W_PER_CORE, 2), dtype=np.int64)
        self.block_tiles = np.zeros((W_PER_CORE, 2), dtype=np.int64)
        for tab in (0, 1):
            nts = a_tiles if tab == 0 else b_tiles
            for w in range(W_PER_CORE):
                self.block_start[w, tab] = len(win_of)
                self.block_tiles[w, tab] = nts[w]
                for _ in range(int(nts[w])):
                    win_of.append(w)
                    table_of.append(tab)
        win_of = np.array(win_of, dtype=np.int64)
        table_of = np.array(table_of, dtype=np.int64)
        T = len(win_of)
        # first/last per (w, tab) block
        first_of = np.zeros(T, dtype=bool)
        last_of = np.zeros(T, dtype=bool)
        for w in range(W_PER_CORE):
            for tab in (0, 1):
                s = int(self.block_start[w, tab])
                n = int(self.block_tiles[w, tab])
                first_of[s] = True
                last_of[s + n - 1] = True

        self.T = T
        self.win_of = win_of
        self.table_of = table_of
        self.first_of = first_of
        self.last_of = last_of
        self.core, self.local, self.win, self.half = core, local, win, half
        self.n_a_tiles = int(a_tiles.sum())

        # gather runs: same-table spans capped at RUN_MAX (tables are the
        # two contiguous passes, so runs only break at the pass boundary)
        runs = []
        t = 0
        while t < T:
            tab = self.table_of[t]
            e = t
            while e < T and self.table_of[e] == tab and e - t < RUN_MAX:
                e += 1
            runs.append((int(tab), t, e - t))
            t = e
        self.runs = runs

        # chunks: <=4-tile pieces within runs
        chunks = []
        for ri, (tab, t0, L) in enumerate(runs):
            t = t0
            while t < t0 + L:
                nt = min(4, t0 + L - t)
                chunks.append((ri, t, nt))
                t += nt
        self.chunks = chunks

    def key(self):
        return (
            tuple(self.win_of.tolist()),
            tuple(self.table_of.tolist()),
        )


def _pack_core(c, S, edge, idx_j):
    """Per-core padded arrays following the common schedule."""
    T = S.T
    perm = np.full(T * P, -1, dtype=np.int64)
    for w in range(W_PER_CORE):
        for tab in (0, 1):
            sel = np.flatnonzero(
                (S.core == c) & (S.win == w) & (S.half == tab)
            )
            n = len(sel)
            s0 = S.block_start[w, tab] * P
            cap = S.block_tiles[w, tab] * P
            assert n <= cap, f"schedule overflow c={c} w={w} tab={tab}"
            perm[s0:s0 + n] = sel

    valid = perm >= 0
    pidx = np.where(valid, perm, 0)

    # idx16 [128, T*8]: per tile-order flattening i=t*128+p -> [i%16, i//16]
    loc = (idx_j[pidx] - S.table_of[np.arange(T * P) // P] * HALF).astype(
        np.int64
    )
    loc[~valid] = 0
    assert (loc >= 0).all() and (loc < HALF).all()
    idx16 = loc.astype(np.int16).reshape(T * 8, 16).T  # [16, T*8]
    idx16 = np.tile(idx16, (8, 1)).copy()              # [128, T*8]

    # chunk-packed edge features and slots
    n_ch = len(S.chunks)
    edgeT = np.zeros((n_ch, D_EDGE, 512), dtype=DT_NP)
    slotp = np.full((n_ch, P, 4), PAD_SLOT, dtype=DT_NP)
    slots_all = (S.local[pidx] % P).astype(DT_NP)
    slots_all[~valid] = PAD_SLOT
    ev = np.zeros((T * P, D_EDGE), dtype=DT_NP)
    ev[valid] = edge[pidx[valid]]
    for k, (ri, t0, nt) in enumerate(S.chunks):
        edgeT[k, :, :nt * P] = ev[t0 * P:(t0 + nt) * P].T
        for t in range(nt):
            slotp[k, :, t] = slots_all[(t0 + t) * P:(t0 + t + 1) * P]
    return edgeT, slotp, idx16


# --------------------------------------------------------------------------
# device program
# --------------------------------------------------------------------------

def _build_program(S):
    T = S.T
    n_ch = len(S.chunks)
    dbg_skip_p1 = os.environ.get("K_SKIP_P1") == "1"
    dbg_skip_gather = os.environ.get("K_SKIP_GATHER") == "1"
    dbg_max_chunks = int(os.environ.get("K_MAX_CHUNKS", "999999"))
    dbg_stages = int(os.environ.get("K_STAGES", "99"))

    nc = bacc.Bacc(
        "TRN2", target_bir_lowering=False, debug=False, num_devices=N_CORES,
        num_swdge_queues=N_QUEUES,
    )

    # ---- I/O ----
    nodeT_h = nc.dram_tensor("nodeT", [P, N_PAD], DT, kind="ExternalInput").ap()
    Wn_h = nc.dram_tensor("Wn", [D_NODE, D_HID], DT, kind="ExternalInput").ap()
    We1_h = nc.dram_tensor("We1p", [D_EDGE, D_HID], DT, kind="ExternalInput").ap()
    We2_h = nc.dram_tensor("We2", [D_HID, D_HID], DT, kind="ExternalInput").ap()
    be1_h = nc.dram_tensor("be1c", [P, 1], F32, kind="ExternalInput").ap()
    be2_h = nc.dram_tensor("be2bc", [P, 512], F32, kind="ExternalInput").ap()
    iota_h = nc.dram_tensor("iota", [P, P], DT, kind="ExternalInput").ap()
    edgeT_h = nc.dram_tensor(
        "edgeT", [n_ch, D_EDGE, 512], DT, kind="ExternalInput"
    ).ap()
    slot_h = nc.dram_tensor(
        "slotp", [n_ch, P, 4], DT, kind="ExternalInput"
    ).ap()
    idx16_h = nc.dram_tensor(
        "idx16", [P, T * 8], I16, kind="ExternalInput"
    ).ap()
    out_h = nc.dram_tensor(
        "out", [W_PER_CORE * P, D_HID], F32, kind="ExternalOutput"
    ).ap()

    msg_h = [
        nc.dram_tensor("msgA", [HALF, D_HID], DT).ap(),
        nc.dram_tensor("msgB", [HALF, D_HID], DT).ap(),
    ]

    LR = mybir.ActivationFunctionType.Prelu

    with tile.TileContext(nc) as tc:
        with tc.tile_pool(name="consts", bufs=1) as cpool:
            Wn_sb = cpool.tile([D_NODE, D_HID], DT)
            nc.sync.dma_start(Wn_sb[:], Wn_h[:])
            We1_sb = cpool.tile([D_EDGE, D_HID], DT)
            nc.sync.dma_start(We1_sb[:], We1_h[:])
            We2_sb = cpool.tile([D_HID, D_HID], DT)
            nc.sync.dma_start(We2_sb[:], We2_h[:])
            be1_sb = cpool.tile([P, 1], F32)
            nc.sync.dma_start(be1_sb[:], be1_h[:])
            be2_sb = cpool.tile([P, 512], F32)
            nc.sync.dma_start(be2_sb[:], be2_h[:])
            iota_sb = cpool.tile([P, P], DT)
            nc.sync.dma_start(iota_sb[:], iota_h[:])

            # ---- phase 1: msg = node @ Wn (slabs of 512 rows; A half first) ----
            p1_stores = [[], []]   # per table
            with (
                tc.tile_pool(name="p1_in", bufs=3) as p1in,
                tc.tile_pool(name="p1_stage", bufs=3) as p1st,
                tc.tile_pool(name="p1_psum", bufs=2, space="PSUM") as p1ps,
            ):
                for g in range(0 if dbg_skip_p1 else N_SLABS):
                    nt_sb = p1in.tile([P, 512], DT, tag="nodeT")
                    nc.sync.dma_start(
                        nt_sb[:], nodeT_h[:, g * 512:(g + 1) * 512]
                    )
                    ps = p1ps.tile([P, 512], F32, tag="p1ps")
                    for t in range(4):
                        nc.tensor.matmul(
                            ps[:, t * P:(t + 1) * P],
                            lhsT=nt_sb[:, t * P:(t + 1) * P],
                            rhs=Wn_sb[:],
                            start=True,
                            stop=True,
                        )
                    stage = p1st.tile([P, 512], DT, tag="p1stage")
                    if g % 2 == 0:
                        nc.vector.tensor_copy(stage[:], ps[:])
                    else:
                        nc.scalar.activation(
                            stage[:], ps[:],
                            mybir.ActivationFunctionType.Copy,
                        )
                    tab = 0 if g < HALF_SLABS else 1
                    r0 = (g % HALF_SLABS) * 512
                    dst = msg_h[tab][r0:r0 + 512, :].rearrange(
                        "(t p) f -> p t f", p=P
                    )
                    srcap = stage[:].rearrange("p (t f) -> p t f", t=4)
                    st_inst = nc.sync.dma_start(dst, srcap)
                    p1_stores[tab].append(st_inst.ins)

            # ---- phase 2: pass A then pass B ----
            with (
                tc.tile_pool(name="p2_in", bufs=6) as p2in,
                tc.tile_pool(name="p2_g", bufs=6) as p2g,
                tc.tile_pool(name="p2_mid", bufs=3) as p2mid,
                tc.tile_pool(name="p2_acc", bufs=1) as accp,
                tc.tile_pool(name="h1_psum", bufs=2, space="PSUM") as h1ps,
                tc.tile_pool(name="h2_psum", bufs=2, space="PSUM") as h2ps,
                tc.tile_pool(name="out_psum", bufs=2, space="PSUM") as outps,
                tc.tile_pool(name="out_stage", bufs=3) as outst,
            ):
                cur_out = {}
                acc = {}

                chunks_by_run = {}
                for k, (ri, t0, nt) in enumerate(S.chunks):
                    chunks_by_run.setdefault(ri, []).append((k, t0, nt))

                first_run_of_tab = {}
                for ri, (tab, rt0, L) in enumerate(S.runs):
                    if tab not in first_run_of_tab:
                        first_run_of_tab[tab] = ri

                for ri, (tab, rt0, L) in enumerate(S.runs):
                    ixr = p2in.tile(
                        [P, RUN_MAX * 8], I16, tag="ix", name=f"ix_r{ri}"
                    )
                    nc.sync.dma_start(
                        ixr[:, :L * 8],
                        idx16_h[:, rt0 * 8:(rt0 + L) * 8],
                    )
                    G = p2g.tile(
                        [P, RUN_MAX * P], DT, tag="G", name=f"G_r{ri}"
                    )
                    if dbg_skip_gather:
                        nc.gpsimd.memset(G[:, :L * P], 0.5)
                    else:
                        g_inst = nc.gpsimd.dma_gather(
                            G[:, :L * P].rearrange("p (g f) -> p g f", f=P),
                            msg_h[tab][:],
                            ixr[:, :L * 8],
                            num_idxs=L * P,
                            num_idxs_reg=L * P,
                            elem_size=P,
                            elem_step=P,
                            queue_num=ri % N_QUEUES,
                        )
                        if ri == first_run_of_tab.get(tab):
                            for st in p1_stores[tab]:
                                add_dep_helper(
                                    g_inst.ins, st, sync=True,
                                    reason=f"gather after msg{tab} stores",
                                )

                    for (k, t0, nt) in chunks_by_run[ri]:
                        if k >= dbg_max_chunks:
                            continue
                        ncols = nt * P
                        goff = (t0 - rt0) * P

                        et_sb = p2in.tile([D_EDGE, 512], DT, tag="edgeT")
                        nc.sync.dma_start(
                            et_sb[:, :ncols], edgeT_h[k, :, :ncols]
                        )
                        sl_sb = p2in.tile([P, 4], DT, tag="slot")
                        nc.sync.dma_start(sl_sb[:, :nt], slot_h[k, :, :nt])

                        # h1 = lrelu(edge @ We1 + be1), feature-major [h x e]
                        ps1 = h1ps.tile([P, 512], F32, tag="h1ps")
                        nc.tensor.matmul(
                            ps1[:, :ncols],
                            lhsT=We1_sb[:],
                            rhs=et_sb[:, :ncols],
                            start=True,
                            stop=True,
                        )
                        h1f = p2mid.tile([P, 512], DT, tag="h1f")
                        if dbg_stages >= 2:
                            nc.scalar.activation(
                                h1f[:, :ncols], ps1[:, :ncols], LR,
                                bias=be1_sb[:], scale=1.0, alpha=NEG_SLOPE,
                            )
                        else:
                            nc.vector.tensor_copy(h1f[:, :ncols], ps1[:, :ncols])
                        if dbg_stages < 3:
                            continue

                        # h2 = h1.T @ We2 + be2, edge-major [e x h]
                        ps2 = h2ps.tile([P, 512], F32, tag="h2ps")
                        for t in range(nt):
                            nc.tensor.matmul(
                                ps2[:, t * P:(t + 1) * P],
                                lhsT=h1f[:, t * P:(t + 1) * P],
                                rhs=We2_sb[:],
                                start=True,
                                stop=True,
                            )
                        nc.vector.tensor_tensor(
                            ps2[:, :ncols], in0=ps2[:, :ncols],
                            in1=be2_sb[:, :ncols], op=mybir.AluOpType.add,
                        )
                        eh = p2mid.tile([P, 512], DT, tag="eh")
                        if dbg_stages >= 4:
                            nc.scalar.activation(
                                eh[:, :ncols], ps2[:, :ncols], LR,
                                scale=1.0, alpha=NEG_SLOPE,
                            )
                        else:
                            nc.vector.tensor_copy(eh[:, :ncols], ps2[:, :ncols])
                        if dbg_stages < 5:
                            continue

                        # onehot[e, s] = (slot[e] == s)
                        oh = p2mid.tile([P, 512], DT, tag="oh")
                        if dbg_stages >= 5:
                            for t in range(nt):
                                nc.vector.tensor_tensor(
                                    oh[:, t * P:(t + 1) * P],
                                    in0=sl_sb[:, t:t + 1].to_broadcast([P, P]),
                                    in1=iota_sb[:],
                                    op=mybir.AluOpType.is_equal,
                                )
                        else:
                            nc.gpsimd.memset(oh[:, :ncols], 0.0)
                        if dbg_stages < 6:
                            continue

                        # product = gathered msg * edge_h
                        pr = p2mid.tile([P, 512], DT, tag="pr")
                        nc.vector.tensor_tensor(
                            pr[:, :ncols],
                            in0=G[:, goff:goff + ncols],
                            in1=eh[:, :ncols],
                            op=mybir.AluOpType.mult,
                        )

                        if dbg_stages < 7:
                            continue
                        # scatter: out_w[s, f] += onehot[:, t].T @ product[:, t]
                        for t in range(nt):
                            i = t0 + t
                            w = int(S.win_of[i])
                            if S.first_of[i]:
                                cur_out[w] = outps.tile(
                                    [P, P], F32, tag="outp",
                                    name=f"outp_w{w}t{tab}"
                                )
                            nc.tensor.matmul(
                                cur_out[w][:],
                                lhsT=oh[:, t * P:(t + 1) * P],
                                rhs=pr[:, t * P:(t + 1) * P],
                                start=bool(S.first_of[i]),
                                stop=bool(S.last_of[i]),
                            )
                            if S.last_of[i]:
                                if tab == 0:
                                    # pass A: stash partial in SBUF
                                    a = accp.tile(
                                        [P, P], F32, tag=f"acc_w{w}",
                                        name=f"acc_w{w}"
                                    )
                                    nc.vector.tensor_copy(a[:], cur_out[w][:])
                                    acc[w] = a
                                else:
                                    # pass B: add pass-A partial, store out
                                    st = outst.tile(
                                        [P, P], F32, tag="outstage",
                                        name=f"outst_w{w}"
                                    )
                                    nc.vector.tensor_tensor(
                                        st[:], in0=cur_out[w][:],
                                        in1=acc[w][:],
                                        op=mybir.AluOpType.add,
                                    )
                                    nc.sync.dma_start(
                                        out_h[w * P:(w + 1) * P, :], st[:]
                                    )
                                del cur_out[w]

    nc.compile()
    return nc


# --------------------------------------------------------------------------
# entry point
# --------------------------------------------------------------------------

def kernel(node, edge, Wn, We1, be1, We2, be2, seg_i, idx_j):
    global LAST_RESULT
    node = np.asarray(node, dtype=np.float32)
    edge = np.asarray(edge, dtype=np.float32)
    Wn = np.asarray(Wn, dtype=np.float32)
    We1 = np.asarray(We1, dtype=np.float32)
    be1 = np.asarray(be1, dtype=np.float32)
    We2 = np.asarray(We2, dtype=np.float32)
    be2 = np.asarray(be2, dtype=np.float32)
    seg_i = np.asarray(seg_i, dtype=np.int32)
    idx_j = np.asarray(idx_j, dtype=np.int32)

    S = Schedule(seg_i.astype(np.int64), idx_j.astype(np.int64))
    key = S.key()
    if key not in _PROGRAM_CACHE:
        _PROGRAM_CACHE[key] = _build_program(S)
    nc = _PROGRAM_CACHE[key]

    nodeT = np.zeros((P, N_PAD), dtype=DT_NP)
    nodeT[:, :N_NODES] = node.T
    iota = np.broadcast_to(np.arange(P, dtype=DT_NP), (P, P)).copy()
    common = {
        "nodeT": nodeT,
        "Wn": Wn.astype(DT_NP),
        "We1p": We1.astype(DT_NP),
        "We2": We2.astype(DT_NP),
        "be1c": be1.reshape(P, 1).copy(),
        "be2bc": np.broadcast_to(
            np.tile(be2, 4), (P, 512)
        ).astype(np.float32).copy(),
        "iota": iota,
    }
    in_maps = []
    for c in range(N_CORES):
        edgeT, slotp, idx16 = _pack_core(c, S, edge, idx_j)
        m = dict(common)
        m["edgeT"] = edgeT
        m["slotp"] = slotp
        m["idx16"] = idx16
        in_maps.append(m)

    if TRACE:
        _ensure_ntff_hook()
    res = run_bass_kernel_spmd(
        nc, in_maps, list(range(N_CORES)), trace=TRACE
    )
    LAST_RESULT = res
    out = np.concatenate(
        [res.results[c]["out"][:NPC] for c in range(N_CORES)], axis=0
    )
    return out.astype(np.float32)


# revision 6
# speedup vs baseline: 1.3921x; 1.0029x over previous
"""Trainium2 Bass kernel for GNN message passing.

Computes, for full inputs:
    edge_h = lrelu(lrelu(edge @ We1 + be1) @ We2 + be2)        # [E, 128]
    out    = segment_sum((node @ Wn)[idx_j] * edge_h, seg_i)   # [N, 128]

Strategy (8 NeuronCores, SPMD single program):
  - Shard edges by DESTINATION range (N/8 = 6250 nodes per core). Each core
    produces its own output slice -> no collectives; host concatenates.
  - Phase 1 (replicated on every core): msg = node @ Wn written to two
    internal DRAM tables (halves, so gather indices fit int16), row-major
    fp16 (256B rows -> full-rate gather). msgA slabs are written first so
    pass-A gathers can start while msgB is still being computed.
  - Phase 2, two passes: pass A processes every window's msgA-sourced edges
    (gathers read only msgA), pass B the msgB-sourced ones. Per 128-edge
    tile: edge MLP via PE matmuls, LeakyReLU (Prelu) on the scalar engine,
    one-hot(slot) on DVE, scatter-add = one-hot matmul accumulated in a
    PSUM window [128 slots x 128 feat]. Pass-A windows flush to SBUF
    accumulators; pass-B flush adds the accumulator and stores to DRAM.
  - Gathers (InstDMAGatherAnt) are issued round-robin on 4 SWDGE queues
    (independent Q7 core pairs) with 6 G buffers, so up to 4 descriptor
    generations overlap: ~3-4 ns/idx instead of ~9 ns/idx single-queue.

The tile->window/table schedule is data-dependent and baked into the
program at build time (all cores share it; per-core data is padded to the
common schedule). Programs are cached per schedule within the process.
"""

import os
import sys
import types

import numpy as np

import concourse.bass as bass
import concourse.tile as tile
from concourse.tile import add_dep_helper
from concourse import bacc, mybir
from concourse.bass_utils import run_bass_kernel_spmd

# ---- problem constants (hardcoded per spec) ----
N_NODES = 50000
D_NODE = 128
D_EDGE = 32
D_HID = 128
N_CORES = 8
NPC = N_NODES // N_CORES          # nodes per core = 6250
P = 128                           # partitions
W_PER_CORE = (NPC + P - 1) // P   # 49 windows per core
NEG_SLOPE = 0.01
PAD_SLOT = 300.0                  # slot value that never matches iota 0..127

# node rows padded to a multiple of 1024 so the two msg halves split evenly
N_PAD = ((N_NODES + 1023) // 1024) * 1024    # 50176
N_SLABS = N_PAD // 512                       # 98
HALF = N_PAD // 2                            # 25088 rows per msg table
HALF_SLABS = N_SLABS // 2                    # 49

RUN_MAX = 8          # max tiles per dma_gather (1024 idxs; ring cap, >1920 wedges)
N_QUEUES = 4         # SWDGE queues (Q7 core pairs) for parallel desc-gen

F32 = mybir.dt.float32
F16 = mybir.dt.float16
I16 = mybir.dt.int16
# data dtype for matmul operands / msg tables (fp16 default; K_DT=f32 to revert)
DT = F32 if os.environ.get("K_DT") == "f32" else F16
DT_NP = np.float32 if os.environ.get("K_DT") == "f32" else np.float16

TRACE = False
LAST_RESULT = None

_PROGRAM_CACHE = {}


def _ensure_ntff_hook():
    """Provide antenv.axon_hooks if this image's antenv lacks it, and
    register the ctypes NTFF profiling hook so trace=True works."""
    try:
        from antenv.axon_hooks import get_axon_ntff_profile_hook  # noqa: F401
        return
    except ImportError:
        pass
    mod = types.ModuleType("antenv.axon_hooks")
    _hook = [None]
    mod.set_axon_ntff_profile_hook = lambda h: _hook.__setitem__(0, h)
    mod.get_axon_ntff_profile_hook = lambda: _hook[0]
    sys.modules["antenv.axon_hooks"] = mod
    import antenv

    antenv.axon_hooks = mod
    try:
        from trn_agent_boot.trn_boot import _ntff_profile_via_ctypes

        mod.set_axon_ntff_profile_hook(
            _ntff_profile_via_ctypes("/opt/axon/libaxon_pjrt.so")
        )
    except Exception:
        pass


# --------------------------------------------------------------------------
# host-side schedule + packing
# --------------------------------------------------------------------------

def cdiv(a, b):
    return (a + b - 1) // b


class Schedule:
    """Common (all-core) static schedule baked into the program.

    Tile sequence = pass A (all windows, table 0) then pass B (table 1).
    """

    def __init__(self, seg_i, idx_j):
        core = seg_i // NPC
        local = seg_i - core * NPC
        win = local // P
        half = (idx_j >= HALF).astype(np.int64)

        cnt = np.zeros((2, N_CORES, W_PER_CORE), dtype=np.int64)
        for c in range(N_CORES):
            m = core == c
            for h in (0, 1):
                cnt[h, c] = np.bincount(
                    win[m & (half == h)], minlength=W_PER_CORE
                )
        a_tiles = np.maximum(cdiv(cnt[0].max(axis=0), P), 1)
        b_tiles = np.maximum(cdiv(cnt[1].max(axis=0), P), 1)

        # tile sequence: pass A (tab 0) then pass B (tab 1)
        win_of, table_of = [], []
        self.block_start = np.zeros((W_PER_CORE, 2), dtype=np.int64)
        self.block_tiles = np.zeros((W_PER_CORE, 2), dtype=np.int64)
        for tab in (0, 1):
            nts = a_tiles if tab == 0 else b_tiles
            for w in range(W_PER_CORE):
                self.block_start[w, tab] = len(win_of)
                self.block_tiles[w, tab] = nts[w]
                for _ in range(int(nts[w])):
                    win_of.append(w)
                    table_of.append(tab)
        win_of = np.array(win_of, dtype=np.int64)
        table_of = np.array(table_of, dtype=np.int64)
        T = len(win_of)
        # first/last per (w, tab) block
        first_of = np.zeros(T, dtype=bool)
        last_of = np.zeros(T, dtype=bool)
        for w in range(W_PER_CORE):
            for tab in (0, 1):
                s = int(self.block_start[w, tab])
                n = int(self.block_tiles[w, tab])
                first_of[s] = True
                last_of[s + n - 1] = True

        self.T = T
        self.win_of = win_of
        self.table_of = table_of
        self.first_of = first_of
        self.last_of = last_of
        self.core, self.local, self.win, self.half = core, local, win, half
        self.n_a_tiles = int(a_tiles.sum())

        # gather runs: same-table spans capped at RUN_MAX (tables are the
        # two contiguous passes, so runs only break at the pass boundary)
        runs = []
        t = 0
        while t < T:
            tab = self.table_of[t]
            e = t
            while e < T and self.table_of[e] == tab and e - t < RUN_MAX:
                e += 1
            runs.append((int(tab), t, e - t))
            t = e
        self.runs = runs

        # chunks: <=4-tile pieces within runs
        chunks = []
        for ri, (tab, t0, L) in enumerate(runs):
            t = t0
            while t < t0 + L:
                nt = min(4, t0 + L - t)
                chunks.append((ri, t, nt))
                t += nt
        self.chunks = chunks

    def key(self):
        return (
            tuple(self.win_of.tolist()),
            tuple(self.table_of.tolist()),
        )


def _pack_core(c, S, edge, idx_j):
    """Per-core padded arrays following the common schedule."""
    T = S.T
    perm = np.full(T * P, -1, dtype=np.int64)
    for w in range(W_PER_CORE):
        for tab in (0, 1):
            sel = np.flatnonzero(
                (S.core == c) & (S.win == w) & (S.half == tab)
            )
            n = len(sel)
            s0 = S.block_start[w, tab] * P
            cap = S.block_tiles[w, tab] * P
            assert n <= cap, f"schedule overflow c={c} w={w} tab={tab}"
            perm[s0:s0 + n] = sel

    valid = perm >= 0
    pidx = np.where(valid, perm, 0)

    # idx16 [128, T*8]: per tile-order flattening i=t*128+p -> [i%16, i//16]
    loc = (idx_j[pidx] - S.table_of[np.arange(T * P) // P] * HALF).astype(
        np.int64
    )
    loc[~valid] = 0
    assert (loc >= 0).all() and (loc < HALF).all()
    idx16 = loc.astype(np.int16).reshape(T * 8, 16).T  # [16, T*8]
    idx16 = np.tile(idx16, (8, 1)).copy()              # [128, T*8]

    # chunk-packed edge features and slots
    n_ch = len(S.chunks)
    edgeT = np.zeros((n_ch, D_EDGE, 512), dtype=DT_NP)
    slotp = np.full((n_ch, P, 4), PAD_SLOT, dtype=DT_NP)
    slots_all = (S.local[pidx] % P).astype(DT_NP)
    slots_all[~valid] = PAD_SLOT
    ev = np.zeros((T * P, D_EDGE), dtype=DT_NP)
    ev[valid] = edge[pidx[valid]]
    for k, (ri, t0, nt) in enumerate(S.chunks):
        edgeT[k, :, :nt * P] = ev[t0 * P:(t0 + nt) * P].T
        for t in range(nt):
            slotp[k, :, t] = slots_all[(t0 + t) * P:(t0 + t + 1) * P]
    return edgeT, slotp, idx16


# --------------------------------------------------------------------------
# device program
# --------------------------------------------------------------------------

def _build_program(S):
    T = S.T
    n_ch = len(S.chunks)
    dbg_skip_p1 = os.environ.get("K_SKIP_P1") == "1"
    dbg_skip_gather = os.environ.get("K_SKIP_GATHER") == "1"
    dbg_max_chunks = int(os.environ.get("K_MAX_CHUNKS", "999999"))
    dbg_stages = int(os.environ.get("K_STAGES", "99"))

    nc = bacc.Bacc(
        "TRN2", target_bir_lowering=False, debug=False, num_devices=N_CORES,
        num_swdge_queues=N_QUEUES,
    )

    # ---- I/O ----
    nodeT_h = nc.dram_tensor("nodeT", [P, N_PAD], DT, kind="ExternalInput").ap()
    Wn_h = nc.dram_tensor("Wn", [D_NODE, D_HID], DT, kind="ExternalInput").ap()
    We1_h = nc.dram_tensor("We1p", [D_EDGE, D_HID], DT, kind="ExternalInput").ap()
    We2_h = nc.dram_tensor("We2", [D_HID, D_HID], DT, kind="ExternalInput").ap()
    be1_h = nc.dram_tensor("be1c", [P, 1], F32, kind="ExternalInput").ap()
    be2_h = nc.dram_tensor("be2bc", [P, 512], F32, kind="ExternalInput").ap()
    iota_h = nc.dram_tensor("iota", [P, P], DT, kind="ExternalInput").ap()
    edgeT_h = nc.dram_tensor(
        "edgeT", [n_ch, D_EDGE, 512], DT, kind="ExternalInput"
    ).ap()
    slot_h = nc.dram_tensor(
        "slotp", [n_ch, P, 4], DT, kind="ExternalInput"
    ).ap()
    idx16_h = nc.dram_tensor(
        "idx16", [P, T * 8], I16, kind="ExternalInput"
    ).ap()
    out_h = nc.dram_tensor(
        "out", [W_PER_CORE * P, D_HID], F32, kind="ExternalOutput"
    ).ap()

    msg_h = [
        nc.dram_tensor("msgA", [HALF, D_HID], DT).ap(),
        nc.dram_tensor("msgB", [HALF, D_HID], DT).ap(),
    ]

    LR = mybir.ActivationFunctionType.Prelu

    with tile.TileContext(nc) as tc:
        with tc.tile_pool(name="consts", bufs=1) as cpool:
            Wn_sb = cpool.tile([D_NODE, D_HID], DT)
            nc.sync.dma_start(Wn_sb[:], Wn_h[:])
            We1_sb = cpool.tile([D_EDGE, D_HID], DT)
            nc.sync.dma_start(We1_sb[:], We1_h[:])
            We2_sb = cpool.tile([D_HID, D_HID], DT)
            nc.sync.dma_start(We2_sb[:], We2_h[:])
            be1_sb = cpool.tile([P, 1], F32)
            nc.sync.dma_start(be1_sb[:], be1_h[:])
            be2_sb = cpool.tile([P, 512], F32)
            nc.sync.dma_start(be2_sb[:], be2_h[:])
            iota_sb = cpool.tile([P, P], DT)
            nc.sync.dma_start(iota_sb[:], iota_h[:])

            # ---- phase 1: msg = node @ Wn (1024-col slabs; A half first) ----
            # slab list: per table, 24 x 1024 + 1 x 512 (HALF = 25088 rows)
            slabs = []
            for tab in (0, 1):
                for k in range(24):
                    slabs.append((tab, k * 1024, 1024))
                slabs.append((tab, 24 * 1024, 512))
            p1_stores = [[], []]   # per table
            with (
                tc.tile_pool(name="p1_in", bufs=3) as p1in,
                tc.tile_pool(name="p1_stage", bufs=3) as p1st,
                tc.tile_pool(name="p1_psum", bufs=2, space="PSUM") as p1ps,
            ):
                for g, (tab, r0, ncols) in enumerate(
                    [] if dbg_skip_p1 else slabs
                ):
                    col0 = tab * HALF + r0
                    nt_sb = p1in.tile([P, 1024], DT, tag="nodeT")
                    nc.sync.dma_start(
                        nt_sb[:, :ncols], nodeT_h[:, col0:col0 + ncols]
                    )
                    ps = p1ps.tile([P, 1024], F32, tag="p1ps")
                    for t in range(ncols // P):
                        nc.tensor.matmul(
                            ps[:, t * P:(t + 1) * P],
                            lhsT=nt_sb[:, t * P:(t + 1) * P],
                            rhs=Wn_sb[:],
                            start=True,
                            stop=True,
                        )
                    stage = p1st.tile([P, 1024], DT, tag="p1stage")
                    if g % 2 == 0:
                        nc.vector.tensor_copy(stage[:, :ncols], ps[:, :ncols])
                    else:
                        nc.scalar.activation(
                            stage[:, :ncols], ps[:, :ncols],
                            mybir.ActivationFunctionType.Copy,
                        )
                    dst = msg_h[tab][r0:r0 + ncols, :].rearrange(
                        "(t p) f -> p t f", p=P
                    )
                    srcap = stage[:, :ncols].rearrange(
                        "p (t f) -> p t f", t=ncols // P
                    )
                    st_inst = nc.sync.dma_start(dst, srcap)
                    p1_stores[tab].append(st_inst.ins)

            # ---- phase 2: pass A then pass B ----
            with (
                tc.tile_pool(name="p2_in", bufs=6) as p2in,
                tc.tile_pool(name="p2_g", bufs=6) as p2g,
                tc.tile_pool(name="p2_mid", bufs=3) as p2mid,
                tc.tile_pool(name="p2_acc", bufs=1) as accp,
                tc.tile_pool(name="h1_psum", bufs=2, space="PSUM") as h1ps,
                tc.tile_pool(name="h2_psum", bufs=2, space="PSUM") as h2ps,
                tc.tile_pool(name="out_psum", bufs=2, space="PSUM") as outps,
                tc.tile_pool(name="out_stage", bufs=3) as outst,
            ):
                cur_out = {}
                acc = {}

                chunks_by_run = {}
                for k, (ri, t0, nt) in enumerate(S.chunks):
                    chunks_by_run.setdefault(ri, []).append((k, t0, nt))

                first_run_of_tab = {}
                for ri, (tab, rt0, L) in enumerate(S.runs):
                    if tab not in first_run_of_tab:
                        first_run_of_tab[tab] = ri

                # ix loads batched: one DMA covers IXG consecutive runs
                IXG = 4
                ix_tiles = {}   # ri -> (tile, col offset)
                cur_ix = None
                for ri, (tab, rt0, L) in enumerate(S.runs):
                    if ri % IXG == 0:
                        span_t0 = rt0
                        span_L = sum(
                            S.runs[j][2]
                            for j in range(ri, min(ri + IXG, len(S.runs)))
                        )
                        cur_ix = p2in.tile(
                            [P, IXG * RUN_MAX * 8], I16, tag="ix",
                            name=f"ix_g{ri // IXG}"
                        )
                        nc.sync.dma_start(
                            cur_ix[:, :span_L * 8],
                            idx16_h[:, span_t0 * 8:(span_t0 + span_L) * 8],
                        )
                        cur_ix_t0 = span_t0
                    ix_tiles[ri] = (cur_ix, (rt0 - cur_ix_t0) * 8)

                for ri, (tab, rt0, L) in enumerate(S.runs):
                    ixr, ixoff = ix_tiles[ri]
                    G = p2g.tile(
                        [P, RUN_MAX * P], DT, tag="G", name=f"G_r{ri}"
                    )
                    if dbg_skip_gather:
                        nc.gpsimd.memset(G[:, :L * P], 0.5)
                    else:
                        g_inst = nc.gpsimd.dma_gather(
                            G[:, :L * P].rearrange("p (g f) -> p g f", f=P),
                            msg_h[tab][:],
                            ixr[:, ixoff:ixoff + L * 8],
                            num_idxs=L * P,
                            num_idxs_reg=L * P,
                            elem_size=P,
                            elem_step=P,
                            queue_num=ri % N_QUEUES,
                        )
                        if ri == first_run_of_tab.get(tab):
                            for st in p1_stores[tab]:
                                add_dep_helper(
                                    g_inst.ins, st, sync=True,
                                    reason=f"gather after msg{tab} stores",
                                )

                    for (k, t0, nt) in chunks_by_run[ri]:
                        if k >= dbg_max_chunks:
                            continue
                        ncols = nt * P
                        goff = (t0 - rt0) * P

                        et_sb = p2in.tile([D_EDGE, 512], DT, tag="edgeT")
                        nc.sync.dma_start(
                            et_sb[:, :ncols], edgeT_h[k, :, :ncols]
                        )
                        sl_sb = p2in.tile([P, 4], DT, tag="slot")
                        nc.sync.dma_start(sl_sb[:, :nt], slot_h[k, :, :nt])

                        # h1 = lrelu(edge @ We1 + be1), feature-major [h x e]
                        ps1 = h1ps.tile([P, 512], F32, tag="h1ps")
                        nc.tensor.matmul(
                            ps1[:, :ncols],
                            lhsT=We1_sb[:],
                            rhs=et_sb[:, :ncols],
                            start=True,
                            stop=True,
                        )
                        h1f = p2mid.tile([P, 512], DT, tag="h1f")
                        if dbg_stages >= 2:
                            nc.scalar.activation(
                                h1f[:, :ncols], ps1[:, :ncols], LR,
                                bias=be1_sb[:], scale=1.0, alpha=NEG_SLOPE,
                            )
                        else:
                            nc.vector.tensor_copy(h1f[:, :ncols], ps1[:, :ncols])
                        if dbg_stages < 3:
                            continue

                        # h2 = h1.T @ We2 + be2, edge-major [e x h]
                        ps2 = h2ps.tile([P, 512], F32, tag="h2ps")
                        for t in range(nt):
                            nc.tensor.matmul(
                                ps2[:, t * P:(t + 1) * P],
                                lhsT=h1f[:, t * P:(t + 1) * P],
                                rhs=We2_sb[:],
                                start=True,
                                stop=True,
                            )
                        nc.vector.tensor_tensor(
                            ps2[:, :ncols], in0=ps2[:, :ncols],
                            in1=be2_sb[:, :ncols], op=mybir.AluOpType.add,
                        )
                        eh = p2mid.tile([P, 512], DT, tag="eh")
                        if dbg_stages >= 4:
                            nc.scalar.activation(
                                eh[:, :ncols], ps2[:, :ncols], LR,
                                scale=1.0, alpha=NEG_SLOPE,
                            )
                        else:
                            nc.vector.tensor_copy(eh[:, :ncols], ps2[:, :ncols])
                        if dbg_stages < 5:
                            continue

                        # onehot[e, s] = (slot[e] == s), all nt tiles in one op
                        oh = p2mid.tile([P, 512], DT, tag="oh")
                        if dbg_stages >= 5:
                            nc.vector.tensor_tensor(
                                oh[:, :ncols].rearrange(
                                    "p (t f) -> p t f", t=nt
                                ),
                                in0=sl_sb[:, :nt].rearrange(
                                    "p (t o) -> p t o", o=1
                                ).to_broadcast([P, nt, P]),
                                in1=iota_sb[:].rearrange(
                                    "p (o f) -> p o f", o=1
                                ).to_broadcast([P, nt, P]),
                                op=mybir.AluOpType.is_equal,
                            )
                        else:
                            nc.gpsimd.memset(oh[:, :ncols], 0.0)
                        if dbg_stages < 6:
                            continue

                        # product = gathered msg * edge_h
                        pr = p2mid.tile([P, 512], DT, tag="pr")
                        nc.vector.tensor_tensor(
                            pr[:, :ncols],
                            in0=G[:, goff:goff + ncols],
                            in1=eh[:, :ncols],
                            op=mybir.AluOpType.mult,
                        )

                        if dbg_stages < 7:
                            continue
                        # scatter: out_w[s, f] += onehot[:, t].T @ product[:, t]
                        for t in range(nt):
                            i = t0 + t
                            w = int(S.win_of[i])
                            if S.first_of[i]:
                                cur_out[w] = outps.tile(
                                    [P, P], F32, tag="outp",
                                    name=f"outp_w{w}t{tab}"
                                )
                            nc.tensor.matmul(
                                cur_out[w][:],
                                lhsT=oh[:, t * P:(t + 1) * P],
                                rhs=pr[:, t * P:(t + 1) * P],
                                start=bool(S.first_of[i]),
                                stop=bool(S.last_of[i]),
                            )
                            if S.last_of[i]:
                                if tab == 0:
                                    # pass A: stash partial in SBUF
                                    a = accp.tile(
                                        [P, P], F32, tag=f"acc_w{w}",
                                        name=f"acc_w{w}"
                                    )
                                    nc.vector.tensor_copy(a[:], cur_out[w][:])
                                    acc[w] = a
                                else:
                                    # pass B: add pass-A partial, store out
                                    st = outst.tile(
                                        [P, P], F32, tag="outstage",
                                        name=f"outst_w{w}"
                                    )
                                    nc.vector.tensor_tensor(
                                        st[:], in0=cur_out[w][:],
                                        in1=acc[w][:],
                                        op=mybir.AluOpType.add,
                                    )
                                    nc.sync.dma_start(
                                        out_h[w * P:(w + 1) * P, :], st[:]
                                    )
                                del cur_out[w]

    nc.compile()
    return nc


# --------------------------------------------------------------------------
# entry point
# --------------------------------------------------------------------------

def kernel(node, edge, Wn, We1, be1, We2, be2, seg_i, idx_j):
    global LAST_RESULT
    node = np.asarray(node, dtype=np.float32)
    edge = np.asarray(edge, dtype=np.float32)
    Wn = np.asarray(Wn, dtype=np.float32)
    We1 = np.asarray(We1, dtype=np.float32)
    be1 = np.asarray(be1, dtype=np.float32)
    We2 = np.asarray(We2, dtype=np.float32)
    be2 = np.asarray(be2, dtype=np.float32)
    seg_i = np.asarray(seg_i, dtype=np.int32)
    idx_j = np.asarray(idx_j, dtype=np.int32)

    S = Schedule(seg_i.astype(np.int64), idx_j.astype(np.int64))
    key = S.key()
    if key not in _PROGRAM_CACHE:
        _PROGRAM_CACHE[key] = _build_program(S)
    nc = _PROGRAM_CACHE[key]

    nodeT = np.zeros((P, N_PAD), dtype=DT_NP)
    nodeT[:, :N_NODES] = node.T
    iota = np.broadcast_to(np.arange(P, dtype=DT_NP), (P, P)).copy()
    common = {
        "nodeT": nodeT,
        "Wn": Wn.astype(DT_NP),
        "We1p": We1.astype(DT_NP),
        "We2": We2.astype(DT_NP),
        "be1c": be1.reshape(P, 1).copy(),
        "be2bc": np.broadcast_to(
            np.tile(be2, 4), (P, 512)
        ).astype(np.float32).copy(),
        "iota": iota,
    }
    in_maps = []
    for c in range(N_CORES):
        edgeT, slotp, idx16 = _pack_core(c, S, edge, idx_j)
        m = dict(common)
        m["edgeT"] = edgeT
        m["slotp"] = slotp
        m["idx16"] = idx16
        in_maps.append(m)

    if TRACE:
        _ensure_ntff_hook()
    res = run_bass_kernel_spmd(
        nc, in_maps, list(range(N_CORES)), trace=TRACE
    )
    LAST_RESULT = res
    out = np.concatenate(
        [res.results[c]["out"][:NPC] for c in range(N_CORES)], axis=0
    )
    return out.astype(np.float32)


# revision 20
# speedup vs baseline: 1.5737x; 1.1304x over previous
"""Trainium2 Bass kernel for GNN message passing.

Computes, for full inputs:
    edge_h = lrelu(lrelu(edge @ We1 + be1) @ We2 + be2)        # [E, 128]
    out    = segment_sum((node @ Wn)[idx_j] * edge_h, seg_i)   # [N, 128]

Strategy (8 NeuronCores, SPMD single program):
  - Shard edges by DESTINATION range (N/8 = 6250 nodes per core). Each core
    produces its own output slice -> no collectives; host concatenates.
  - Phase 1 (replicated on every core): msg = node @ Wn written to two
    internal DRAM tables (halves, so gather indices fit int16), row-major
    fp16 (256B rows -> full-rate gather). msgA slabs are written first so
    pass-A gathers can start while msgB is still being computed.
  - Phase 2, two passes: pass A processes every window's msgA-sourced edges
    (gathers read only msgA), pass B the msgB-sourced ones. Per 128-edge
    tile: edge MLP via PE matmuls, LeakyReLU (Prelu) on the scalar engine,
    one-hot(slot) on DVE, scatter-add = one-hot matmul accumulated in a
    PSUM window [128 slots x 128 feat]. Pass-A windows flush to SBUF
    accumulators; pass-B flush adds the accumulator and stores to DRAM.
  - Gathers (InstDMAGatherAnt) are issued round-robin on 4 SWDGE queues
    (independent Q7 core pairs) with 6 G buffers, so up to 4 descriptor
    generations overlap: ~3-4 ns/idx instead of ~9 ns/idx single-queue.

The tile->window/table schedule is data-dependent and baked into the
program at build time (all cores share it; per-core data is padded to the
common schedule). Programs are cached per schedule within the process.
"""

import os
import sys
import types

import numpy as np

import concourse.bass as bass
import concourse.tile as tile
from concourse.tile import add_dep_helper
from concourse import bacc, mybir
from concourse.bass_utils import run_bass_kernel_spmd

# ---- problem constants (hardcoded per spec) ----
N_NODES = 50000
D_NODE = 128
D_EDGE = 32
D_HID = 128
N_CORES = 8
NPC = N_NODES // N_CORES          # nodes per core = 6250
P = 128                           # partitions
W_PER_CORE = (NPC + P - 1) // P   # 49 windows per core
NEG_SLOPE = 0.01
PAD_SLOT = 300.0                  # slot value that never matches iota 0..127

# node rows padded to a multiple of 1024 so the two msg halves split evenly
N_PAD = ((N_NODES + 1023) // 1024) * 1024    # 50176
N_SLABS = N_PAD // 512                       # 98
HALF = N_PAD // 2                            # 25088 rows per msg table
HALF_SLABS = N_SLABS // 2                    # 49

RUN_MAX = 8          # max tiles per dma_gather (1024 idxs; ring cap, >1920 wedges)
N_QUEUES = 4         # SWDGE queues (Q7 core pairs) for parallel desc-gen
TPB = HALF // P      # 196: msg table stored partition-major, k = p*TPB + t

F32 = mybir.dt.float32
F16 = mybir.dt.float16
I16 = mybir.dt.int16
# data dtype for matmul operands / msg tables (fp16 default; K_DT=f32 to revert)
DT = F32 if os.environ.get("K_DT") == "f32" else F16
DT_NP = np.float32 if os.environ.get("K_DT") == "f32" else np.float16

TRACE = False
LAST_RESULT = None

_PROGRAM_CACHE = {}


def _ensure_ntff_hook():
    """Provide antenv.axon_hooks if this image's antenv lacks it, and
    register the ctypes NTFF profiling hook so trace=True works."""
    try:
        from antenv.axon_hooks import get_axon_ntff_profile_hook  # noqa: F401
        return
    except ImportError:
        pass
    mod = types.ModuleType("antenv.axon_hooks")
    _hook = [None]
    mod.set_axon_ntff_profile_hook = lambda h: _hook.__setitem__(0, h)
    mod.get_axon_ntff_profile_hook = lambda: _hook[0]
    sys.modules["antenv.axon_hooks"] = mod
    import antenv

    antenv.axon_hooks = mod
    try:
        from trn_agent_boot.trn_boot import _ntff_profile_via_ctypes

        mod.set_axon_ntff_profile_hook(
            _ntff_profile_via_ctypes("/opt/axon/libaxon_pjrt.so")
        )
    except Exception:
        pass


# --------------------------------------------------------------------------
# host-side schedule + packing
# --------------------------------------------------------------------------

def cdiv(a, b):
    return (a + b - 1) // b


class Schedule:
    """Common (all-core) static schedule baked into the program.

    Tile sequence = pass A (all windows, table 0) then pass B (table 1).
    """

    def __init__(self, seg_i, idx_j):
        core = seg_i // NPC
        local = seg_i - core * NPC
        win = local // P
        half = (idx_j >= HALF).astype(np.int64)

        cnt = np.zeros((2, N_CORES, W_PER_CORE), dtype=np.int64)
        for c in range(N_CORES):
            m = core == c
            for h in (0, 1):
                cnt[h, c] = np.bincount(
                    win[m & (half == h)], minlength=W_PER_CORE
                )
        a_tiles = np.maximum(cdiv(cnt[0].max(axis=0), P), 1)
        b_tiles = np.maximum(cdiv(cnt[1].max(axis=0), P), 1)

        # tile sequence: pass A (tab 0) then pass B (tab 1)
        win_of, table_of = [], []
        self.block_start = np.zeros((W_PER_CORE, 2), dtype=np.int64)
        self.block_tiles = np.zeros((W_PER_CORE, 2), dtype=np.int64)
        for tab in (0, 1):
            nts = a_tiles if tab == 0 else b_tiles
            for w in range(W_PER_CORE):
                self.block_start[w, tab] = len(win_of)
                self.block_tiles[w, tab] = nts[w]
                for _ in range(int(nts[w])):
                    win_of.append(w)
                    table_of.append(tab)
        win_of = np.array(win_of, dtype=np.int64)
        table_of = np.array(table_of, dtype=np.int64)
        T = len(win_of)
        # first/last per (w, tab) block
        first_of = np.zeros(T, dtype=bool)
        last_of = np.zeros(T, dtype=bool)
        for w in range(W_PER_CORE):
            for tab in (0, 1):
                s = int(self.block_start[w, tab])
                n = int(self.block_tiles[w, tab])
                first_of[s] = True
                last_of[s + n - 1] = True

        self.T = T
        self.win_of = win_of
        self.table_of = table_of
        self.first_of = first_of
        self.last_of = last_of
        self.core, self.local, self.win, self.half = core, local, win, half
        self.n_a_tiles = int(a_tiles.sum())

        # gather runs: same-table spans capped at RUN_MAX (tables are the
        # two contiguous passes, so runs only break at the pass boundary)
        runs = []
        t = 0
        while t < T:
            tab = self.table_of[t]
            e = t
            while e < T and self.table_of[e] == tab and e - t < RUN_MAX:
                e += 1
            runs.append((int(tab), t, e - t))
            t = e
        self.runs = runs

        # chunks: <=4-tile pieces within runs
        chunks = []
        for ri, (tab, t0, L) in enumerate(runs):
            t = t0
            while t < t0 + L:
                nt = min(4, t0 + L - t)
                chunks.append((ri, t, nt))
                t += nt
        self.chunks = chunks

    def key(self):
        return (
            tuple(self.win_of.tolist()),
            tuple(self.table_of.tolist()),
        )


def _pack_core(c, S, edge, idx_j):
    """Per-core padded arrays following the common schedule."""
    T = S.T
    perm = np.full(T * P, -1, dtype=np.int64)
    for w in range(W_PER_CORE):
        for tab in (0, 1):
            sel = np.flatnonzero(
                (S.core == c) & (S.win == w) & (S.half == tab)
            )
            n = len(sel)
            s0 = S.block_start[w, tab] * P
            cap = S.block_tiles[w, tab] * P
            assert n <= cap, f"schedule overflow c={c} w={w} tab={tab}"
            perm[s0:s0 + n] = sel

    valid = perm >= 0
    pidx = np.where(valid, perm, 0)

    # idx16 [128, T*8]: per tile-order flattening i=t*128+p -> [i%16, i//16]
    loc = (idx_j[pidx] - S.table_of[np.arange(T * P) // P] * HALF).astype(
        np.int64
    )
    loc[~valid] = 0
    assert (loc >= 0).all() and (loc < HALF).all()
    if os.environ.get("K_REMAP") != "0":
        # msg tables partition-major: node j at position (j%128)*TPB + j//128
        loc = (loc % P) * TPB + loc // P
    idx16 = loc.astype(np.int16).reshape(T * 8, 16).T  # [16, T*8]
    idx16 = np.tile(idx16, (8, 1)).copy()              # [128, T*8]

    # chunk-packed edge features and slots
    n_ch = len(S.chunks)
    edgeT = np.zeros((n_ch, D_EDGE, 512), dtype=DT_NP)
    slotp = np.full((n_ch, P, 4), PAD_SLOT, dtype=DT_NP)
    slots_all = (S.local[pidx] % P).astype(DT_NP)
    slots_all[~valid] = PAD_SLOT
    ev = np.zeros((T * P, D_EDGE), dtype=DT_NP)
    ev[valid] = edge[pidx[valid]]
    for k, (ri, t0, nt) in enumerate(S.chunks):
        edgeT[k, :, :nt * P] = ev[t0 * P:(t0 + nt) * P].T
        for t in range(nt):
            slotp[k, :, t] = slots_all[(t0 + t) * P:(t0 + t + 1) * P]
    return edgeT, slotp, idx16


# --------------------------------------------------------------------------
# device program
# --------------------------------------------------------------------------

def _build_program(S):
    T = S.T
    n_ch = len(S.chunks)
    dbg_skip_p1 = os.environ.get("K_SKIP_P1") == "1"
    dbg_skip_gather = os.environ.get("K_SKIP_GATHER") == "1"
    dbg_max_chunks = int(os.environ.get("K_MAX_CHUNKS", "999999"))
    dbg_stages = int(os.environ.get("K_STAGES", "99"))

    nc = bacc.Bacc(
        "TRN2", target_bir_lowering=False, debug=False, num_devices=N_CORES,
        num_swdge_queues=N_QUEUES,
    )

    # ---- I/O ----
    nodeT_h = nc.dram_tensor("nodeT", [P, N_PAD], DT, kind="ExternalInput").ap()
    Wn_h = nc.dram_tensor("Wn", [D_NODE, D_HID], DT, kind="ExternalInput").ap()
    We1_h = nc.dram_tensor("We1p", [D_EDGE, D_HID], DT, kind="ExternalInput").ap()
    We2_h = nc.dram_tensor("We2", [D_HID, D_HID], DT, kind="ExternalInput").ap()
    be1_h = nc.dram_tensor("be1c", [P, 1], F32, kind="ExternalInput").ap()
    be2_h = nc.dram_tensor("be2bc", [P, 512], F32, kind="ExternalInput").ap()
    iota_h = nc.dram_tensor("iota", [P, P], DT, kind="ExternalInput").ap()
    ones_h = nc.dram_tensor("ones1", [1, P], DT, kind="ExternalInput").ap()
    be2r_h = nc.dram_tensor("be2r", [1, 512], DT, kind="ExternalInput").ap()
    edgeT_h = nc.dram_tensor(
        "edgeT", [n_ch, D_EDGE, 512], DT, kind="ExternalInput"
    ).ap()
    slot_h = nc.dram_tensor(
        "slotp", [n_ch, P, 4], DT, kind="ExternalInput"
    ).ap()
    idx16_h = nc.dram_tensor(
        "idx16", [P, T * 8], I16, kind="ExternalInput"
    ).ap()
    out_h = nc.dram_tensor(
        "out", [W_PER_CORE * P, D_HID], F32, kind="ExternalOutput"
    ).ap()

    msg_h = [
        nc.dram_tensor("msgA", [HALF, D_HID], DT).ap(),
        nc.dram_tensor("msgB", [HALF, D_HID], DT).ap(),
    ]

    LR = mybir.ActivationFunctionType.Prelu

    with tile.TileContext(nc) as tc:
        with tc.tile_pool(name="consts", bufs=1) as cpool:
            Wn_sb = cpool.tile([D_NODE, D_HID], DT)
            nc.sync.dma_start(Wn_sb[:], Wn_h[:])
            We1_sb = cpool.tile([D_EDGE, D_HID], DT)
            nc.sync.dma_start(We1_sb[:], We1_h[:])
            We2_sb = cpool.tile([D_HID, D_HID], DT)
            nc.sync.dma_start(We2_sb[:], We2_h[:])
            be1_sb = cpool.tile([P, 1], F32)
            nc.sync.dma_start(be1_sb[:], be1_h[:])
            be2_sb = cpool.tile([P, 512], F32)
            nc.sync.dma_start(be2_sb[:], be2_h[:])
            iota_sb = cpool.tile([P, P], DT)
            nc.sync.dma_start(iota_sb[:], iota_h[:])
            ones_sb = cpool.tile([1, P], DT)
            nc.sync.dma_start(ones_sb[:], ones_h[:])
            be2r_sb = cpool.tile([1, 512], DT)
            nc.sync.dma_start(be2r_sb[:], be2r_h[:])

            # ---- phase 1: msg = node @ Wn (1024-col slabs; A half first) ----
            # slab list: per table, 24 x 1024 + 1 x 512 (HALF = 25088 rows)
            slabs = []
            for tab in (0, 1):
                for k in range(24):
                    slabs.append((tab, k * 1024, 1024))
                slabs.append((tab, 24 * 1024, 512))
            p1_stores = [[], []]   # per table
            with (
                tc.tile_pool(name="p1_in", bufs=3) as p1in,
                tc.tile_pool(name="p1_stage", bufs=3) as p1st,
                tc.tile_pool(name="p1_psum", bufs=2, space="PSUM") as p1ps,
            ):
                for g, (tab, r0, ncols) in enumerate(
                    [] if dbg_skip_p1 else slabs
                ):
                    col0 = tab * HALF + r0
                    nt_sb = p1in.tile([P, 1024], DT, tag="nodeT")
                    nc.sync.dma_start(
                        nt_sb[:, :ncols], nodeT_h[:, col0:col0 + ncols]
                    )
                    ps = p1ps.tile([P, 1024], F32, tag="p1ps")
                    for t in range(ncols // P):
                        nc.tensor.matmul(
                            ps[:, t * P:(t + 1) * P],
                            lhsT=nt_sb[:, t * P:(t + 1) * P],
                            rhs=Wn_sb[:],
                            start=True,
                            stop=True,
                        )
                    stage = p1st.tile([P, 1024], DT, tag="p1stage")
                    if g % 2 == 0:
                        nc.vector.tensor_copy(stage[:, :ncols], ps[:, :ncols])
                    else:
                        nc.scalar.activation(
                            stage[:, :ncols], ps[:, :ncols],
                            mybir.ActivationFunctionType.Copy,
                        )
                    # table is partition-major: slab rows j=r0+t*128+p land at
                    # k = p*TPB + (r0//128 + t) -> contiguous 2KB per partition
                    if os.environ.get("K_REMAP") != "0":
                        t0 = r0 // P
                        dst = msg_h[tab][:].rearrange(
                            "(p t) f -> p t f", t=TPB
                        )[:, t0:t0 + ncols // P, :]
                    else:
                        dst = msg_h[tab][r0:r0 + ncols, :].rearrange(
                            "(t p) f -> p t f", p=P
                        )
                    srcap = stage[:, :ncols].rearrange(
                        "p (t f) -> p t f", t=ncols // P
                    )
                    st_inst = nc.sync.dma_start(dst, srcap)
                    p1_stores[tab].append(st_inst.ins)

            # ---- phase 2: pass A then pass B ----
            with (
                tc.tile_pool(name="p2_in", bufs=6) as p2in,
                tc.tile_pool(name="p2_g", bufs=8) as p2g,
                tc.tile_pool(name="p2_mid", bufs=3) as p2mid,
                tc.tile_pool(name="p2_acc", bufs=1) as accp,
                tc.tile_pool(name="h1_psum", bufs=2, space="PSUM") as h1ps,
                tc.tile_pool(name="h2_psum", bufs=2, space="PSUM") as h2ps,
                tc.tile_pool(name="out_psum", bufs=2, space="PSUM") as outps,
                tc.tile_pool(name="out_stage", bufs=3) as outst,
            ):
                cur_out = {}
                acc = {}

                chunks_by_run = {}
                for k, (ri, t0, nt) in enumerate(S.chunks):
                    chunks_by_run.setdefault(ri, []).append((k, t0, nt))

                first_run_of_tab = {}
                for ri, (tab, rt0, L) in enumerate(S.runs):
                    if tab not in first_run_of_tab:
                        first_run_of_tab[tab] = ri

                # ix loads batched: one DMA covers IXG consecutive runs
                IXG = 4
                ix_tiles = {}   # ri -> (tile, col offset)
                cur_ix = None
                for ri, (tab, rt0, L) in enumerate(S.runs):
                    if ri % IXG == 0:
                        span_t0 = rt0
                        span_L = sum(
                            S.runs[j][2]
                            for j in range(ri, min(ri + IXG, len(S.runs)))
                        )
                        cur_ix = p2in.tile(
                            [P, IXG * RUN_MAX * 8], I16, tag="ix",
                            name=f"ix_g{ri // IXG}"
                        )
                        nc.sync.dma_start(
                            cur_ix[:, :span_L * 8],
                            idx16_h[:, span_t0 * 8:(span_t0 + span_L) * 8],
                        )
                        cur_ix_t0 = span_t0
                    ix_tiles[ri] = (cur_ix, (rt0 - cur_ix_t0) * 8)

                for ri, (tab, rt0, L) in enumerate(S.runs):
                    ixr, ixoff = ix_tiles[ri]
                    G = p2g.tile(
                        [P, RUN_MAX * P], DT, tag="G", name=f"G_r{ri}"
                    )
                    if dbg_skip_gather:
                        nc.gpsimd.memset(G[:, :L * P], 0.5)
                    else:
                        g_inst = nc.gpsimd.dma_gather(
                            G[:, :L * P].rearrange("p (g f) -> p g f", f=P),
                            msg_h[tab][:],
                            ixr[:, ixoff:ixoff + L * 8],
                            num_idxs=L * P,
                            num_idxs_reg=L * P,
                            elem_size=P,
                            elem_step=P,
                            queue_num=ri % N_QUEUES,
                        )
                        if ri == first_run_of_tab.get(tab):
                            for st in p1_stores[tab]:
                                add_dep_helper(
                                    g_inst.ins, st, sync=True,
                                    reason=f"gather after msg{tab} stores",
                                )

                    for (k, t0, nt) in chunks_by_run[ri]:
                        if k >= dbg_max_chunks:
                            continue
                        ncols = nt * P
                        goff = (t0 - rt0) * P

                        et_sb = p2in.tile([D_EDGE, 512], DT, tag="edgeT")
                        nc.sync.dma_start(
                            et_sb[:, :ncols], edgeT_h[k, :, :ncols]
                        )
                        sl_sb = p2in.tile([P, 4], DT, tag="slot")
                        nc.sync.dma_start(sl_sb[:, :nt], slot_h[k, :, :nt])

                        # h1 = lrelu(edge @ We1 + be1), feature-major [h x e]
                        ps1 = h1ps.tile([P, 512], F32, tag="h1ps")
                        nc.tensor.matmul(
                            ps1[:, :ncols],
                            lhsT=We1_sb[:],
                            rhs=et_sb[:, :ncols],
                            start=True,
                            stop=True,
                        )
                        h1f = p2mid.tile([P, 512], DT, tag="h1f")
                        if dbg_stages >= 2:
                            nc.scalar.activation(
                                h1f[:, :ncols], ps1[:, :ncols], LR,
                                bias=be1_sb[:], scale=1.0, alpha=NEG_SLOPE,
                            )
                        else:
                            nc.vector.tensor_copy(h1f[:, :ncols], ps1[:, :ncols])
                        if dbg_stages < 3:
                            continue

                        # h2 = h1.T @ We2 + be2, edge-major [e x h];
                        # the bias lands via a K=1 accumulate matmul (PE, not DVE)
                        ps2 = h2ps.tile([P, 512], F32, tag="h2ps")
                        use_bias_mm = os.environ.get("K_BIAS_MM") != "0"
                        mm2s = []
                        for t in range(nt):
                            mm2s.append(nc.tensor.matmul(
                                ps2[:, t * P:(t + 1) * P],
                                lhsT=h1f[:, t * P:(t + 1) * P],
                                rhs=We2_sb[:],
                                start=True,
                                stop=not use_bias_mm,
                            ))
                        if use_bias_mm:
                            bmm = nc.tensor.matmul(
                                ps2[:, :ncols],
                                lhsT=ones_sb[:],
                                rhs=be2r_sb[:, :ncols],
                                start=False,
                                stop=True,
                            )
                            for m in mm2s:
                                add_dep_helper(
                                    bmm.ins, m.ins, sync=False,
                                    reason="bias accumulate after mm2 blocks",
                                )
                        else:
                            nc.vector.tensor_tensor(
                                ps2[:, :ncols], in0=ps2[:, :ncols],
                                in1=be2_sb[:, :ncols], op=mybir.AluOpType.add,
                            )
                        eh = p2mid.tile([P, 512], DT, tag="eh")
                        if dbg_stages >= 4:
                            nc.scalar.activation(
                                eh[:, :ncols], ps2[:, :ncols], LR,
                                scale=1.0, alpha=NEG_SLOPE,
                            )
                        else:
                            nc.vector.tensor_copy(eh[:, :ncols], ps2[:, :ncols])
                        if dbg_stages < 5:
                            continue

                        # onehot[e, s] = (slot[e] == s), all nt tiles in one op
                        oh = p2mid.tile([P, 512], DT, tag="oh")
                        if dbg_stages >= 5:
                            nc.vector.tensor_tensor(
                                oh[:, :ncols].rearrange(
                                    "p (t f) -> p t f", t=nt
                                ),
                                in0=sl_sb[:, :nt].rearrange(
                                    "p (t o) -> p t o", o=1
                                ).to_broadcast([P, nt, P]),
                                in1=iota_sb[:].rearrange(
                                    "p (o f) -> p o f", o=1
                                ).to_broadcast([P, nt, P]),
                                op=mybir.AluOpType.is_equal,
                            )
                        else:
                            nc.gpsimd.memset(oh[:, :ncols], 0.0)
                        if dbg_stages < 6:
                            continue

                        # product = gathered msg * edge_h
                        pr = p2mid.tile([P, 512], DT, tag="pr")
                        nc.vector.tensor_tensor(
                            pr[:, :ncols],
                            in0=G[:, goff:goff + ncols],
                            in1=eh[:, :ncols],
                            op=mybir.AluOpType.mult,
                        )

                        if dbg_stages < 7:
                            continue
                        # scatter: out_w[s, f] += onehot[:, t].T @ product[:, t]
                        for t in range(nt):
                            i = t0 + t
                            w = int(S.win_of[i])
                            if S.first_of[i]:
                                cur_out[w] = outps.tile(
                                    [P, P], F32, tag="outp",
                                    name=f"outp_w{w}t{tab}"
                                )
                            nc.tensor.matmul(
                                cur_out[w][:],
                                lhsT=oh[:, t * P:(t + 1) * P],
                                rhs=pr[:, t * P:(t + 1) * P],
                                start=bool(S.first_of[i]),
                                stop=bool(S.last_of[i]),
                            )
                            if S.last_of[i]:
                                if tab == 0:
                                    # pass A: stash partial in SBUF (scalar
                                    # engine copy; DVE is the busier one)
                                    a = accp.tile(
                                        [P, P], F32, tag=f"acc_w{w}",
                                        name=f"acc_w{w}"
                                    )
                                    nc.scalar.activation(
                                        a[:], cur_out[w][:],
                                        mybir.ActivationFunctionType.Copy,
                                    )
                                    acc[w] = a
                                else:
                                    # pass B: add pass-A partial, store out
                                    st = outst.tile(
                                        [P, P], F32, tag="outstage",
                                        name=f"outst_w{w}"
                                    )
                                    nc.vector.tensor_tensor(
                                        st[:], in0=cur_out[w][:],
                                        in1=acc[w][:],
                                        op=mybir.AluOpType.add,
                                    )
                                    nc.sync.dma_start(
                                        out_h[w * P:(w + 1) * P, :], st[:]
                                    )
                                del cur_out[w]

    nc.compile()
    return nc


# --------------------------------------------------------------------------
# entry point
# --------------------------------------------------------------------------

def kernel(node, edge, Wn, We1, be1, We2, be2, seg_i, idx_j):
    global LAST_RESULT
    node = np.asarray(node, dtype=np.float32)
    edge = np.asarray(edge, dtype=np.float32)
    Wn = np.asarray(Wn, dtype=np.float32)
    We1 = np.asarray(We1, dtype=np.float32)
    be1 = np.asarray(be1, dtype=np.float32)
    We2 = np.asarray(We2, dtype=np.float32)
    be2 = np.asarray(be2, dtype=np.float32)
    seg_i = np.asarray(seg_i, dtype=np.int32)
    idx_j = np.asarray(idx_j, dtype=np.int32)

    S = Schedule(seg_i.astype(np.int64), idx_j.astype(np.int64))
    key = S.key()
    if key not in _PROGRAM_CACHE:
        _PROGRAM_CACHE[key] = _build_program(S)
    nc = _PROGRAM_CACHE[key]

    nodeT = np.zeros((P, N_PAD), dtype=DT_NP)
    nodeT[:, :N_NODES] = node.T
    iota = np.broadcast_to(np.arange(P, dtype=DT_NP), (P, P)).copy()
    common = {
        "nodeT": nodeT,
        "Wn": Wn.astype(DT_NP),
        "We1p": We1.astype(DT_NP),
        "We2": We2.astype(DT_NP),
        "be1c": be1.reshape(P, 1).copy(),
        "be2bc": np.broadcast_to(
            np.tile(be2, 4), (P, 512)
        ).astype(np.float32).copy(),
        "iota": iota,
        "ones1": np.ones((1, P), dtype=DT_NP),
        "be2r": np.tile(be2, 4).reshape(1, 512).astype(DT_NP),
    }
    in_maps = []
    for c in range(N_CORES):
        edgeT, slotp, idx16 = _pack_core(c, S, edge, idx_j)
        m = dict(common)
        m["edgeT"] = edgeT
        m["slotp"] = slotp
        m["idx16"] = idx16
        in_maps.append(m)

    if TRACE:
        _ensure_ntff_hook()
    res = run_bass_kernel_spmd(
        nc, in_maps, list(range(N_CORES)), trace=TRACE
    )
    LAST_RESULT = res
    out = np.concatenate(
        [res.results[c]["out"][:NPC] for c in range(N_CORES)], axis=0
    )
    return out.astype(np.float32)


# revision 29
# speedup vs baseline: 1.9514x; 1.2400x over previous
"""Trainium2 Bass kernel for GNN message passing.

Computes, for full inputs:
    edge_h = lrelu(lrelu(edge @ We1 + be1) @ We2 + be2)        # [E, 128]
    out    = segment_sum((node @ Wn)[idx_j] * edge_h, seg_i)   # [N, 128]

Strategy (8 NeuronCores, SPMD single program):
  - Shard edges by DESTINATION range (N/8 = 6250 nodes per core). Each core
    produces its own output slice -> no collectives; host concatenates.
  - Phase 1 (replicated on every core): msg = node @ Wn written to two
    internal DRAM tables (halves, so gather indices fit int16), row-major
    fp16 (256B rows -> full-rate gather). msgA slabs are written first so
    pass-A gathers can start while msgB is still being computed.
  - Phase 2, two passes: pass A processes every window's msgA-sourced edges
    (gathers read only msgA), pass B the msgB-sourced ones. Per 128-edge
    tile: edge MLP via PE matmuls, LeakyReLU (Prelu) on the scalar engine,
    one-hot(slot) on DVE, scatter-add = one-hot matmul accumulated in a
    PSUM window [128 slots x 128 feat]. Pass-A windows flush to SBUF
    accumulators; pass-B flush adds the accumulator and stores to DRAM.
  - Gathers (InstDMAGatherAnt) are issued round-robin on 4 SWDGE queues
    (independent Q7 core pairs) with 6 G buffers, so up to 4 descriptor
    generations overlap: ~3-4 ns/idx instead of ~9 ns/idx single-queue.

The tile->window/table schedule is data-dependent and baked into the
program at build time (all cores share it; per-core data is padded to the
common schedule). Programs are cached per schedule within the process.
"""

import os
import sys
import types

import numpy as np

import concourse.bass as bass
import concourse.tile as tile
from concourse.tile import add_dep_helper
from concourse import bacc, mybir
from concourse.bass_utils import run_bass_kernel_spmd

# ---- problem constants (hardcoded per spec) ----
N_NODES = 50000
D_NODE = 128
D_EDGE = 32
D_HID = 128
N_CORES = 8
NPC = N_NODES // N_CORES          # nodes per core = 6250
P = 128                           # partitions
W_PER_CORE = (NPC + P - 1) // P   # 49 windows per core
NEG_SLOPE = 0.01
PAD_SLOT = 300.0                  # slot value that never matches iota 0..127

# node rows padded to a multiple of 1024 so the two msg halves split evenly
N_PAD = ((N_NODES + 1023) // 1024) * 1024    # 50176
N_SLABS = N_PAD // 512                       # 98
HALF = N_PAD // 2                            # 25088 rows per msg table
HALF_SLABS = N_SLABS // 2                    # 49

RUN_MAX = 8          # max tiles per dma_gather (1024 idxs; ring cap, >1920 wedges)
N_QUEUES = 4         # SWDGE queues (Q7 core pairs) for parallel desc-gen
TPB = HALF // P      # 196: msg table stored partition-major, k = p*TPB + t

F32 = mybir.dt.float32
F16 = mybir.dt.float16
I16 = mybir.dt.int16
# data dtype for matmul operands / msg tables (fp16 default; K_DT=f32 to revert)
DT = F32 if os.environ.get("K_DT") == "f32" else F16
DT_NP = np.float32 if os.environ.get("K_DT") == "f32" else np.float16

TRACE = False
LAST_RESULT = None

_PROGRAM_CACHE = {}


def _ensure_ntff_hook():
    """Provide antenv.axon_hooks if this image's antenv lacks it, and
    register the ctypes NTFF profiling hook so trace=True works."""
    try:
        from antenv.axon_hooks import get_axon_ntff_profile_hook  # noqa: F401
        return
    except ImportError:
        pass
    mod = types.ModuleType("antenv.axon_hooks")
    _hook = [None]
    mod.set_axon_ntff_profile_hook = lambda h: _hook.__setitem__(0, h)
    mod.get_axon_ntff_profile_hook = lambda: _hook[0]
    sys.modules["antenv.axon_hooks"] = mod
    import antenv

    antenv.axon_hooks = mod
    try:
        from trn_agent_boot.trn_boot import _ntff_profile_via_ctypes

        mod.set_axon_ntff_profile_hook(
            _ntff_profile_via_ctypes("/opt/axon/libaxon_pjrt.so")
        )
    except Exception:
        pass


# --------------------------------------------------------------------------
# host-side schedule + packing
# --------------------------------------------------------------------------

def cdiv(a, b):
    return (a + b - 1) // b


class Schedule:
    """Common (all-core) static schedule baked into the program.

    Tile sequence = pass A (all windows, table 0) then pass B (table 1).
    """

    def __init__(self, seg_i, idx_j):
        core = seg_i // NPC
        local = seg_i - core * NPC
        win = local // P
        half = (idx_j >= HALF).astype(np.int64)

        cnt = np.zeros((2, N_CORES, W_PER_CORE), dtype=np.int64)
        for c in range(N_CORES):
            m = core == c
            for h in (0, 1):
                cnt[h, c] = np.bincount(
                    win[m & (half == h)], minlength=W_PER_CORE
                )
        a_tiles = np.maximum(cdiv(cnt[0].max(axis=0), P), 1)
        b_tiles = np.maximum(cdiv(cnt[1].max(axis=0), P), 1)

        # tile sequence: pass A (tab 0) then pass B (tab 1)
        win_of, table_of = [], []
        self.block_start = np.zeros((W_PER_CORE, 2), dtype=np.int64)
        self.block_tiles = np.zeros((W_PER_CORE, 2), dtype=np.int64)
        for tab in (0, 1):
            nts = a_tiles if tab == 0 else b_tiles
            for w in range(W_PER_CORE):
                self.block_start[w, tab] = len(win_of)
                self.block_tiles[w, tab] = nts[w]
                for _ in range(int(nts[w])):
                    win_of.append(w)
                    table_of.append(tab)
        win_of = np.array(win_of, dtype=np.int64)
        table_of = np.array(table_of, dtype=np.int64)
        T = len(win_of)
        # first/last per (w, tab) block
        first_of = np.zeros(T, dtype=bool)
        last_of = np.zeros(T, dtype=bool)
        for w in range(W_PER_CORE):
            for tab in (0, 1):
                s = int(self.block_start[w, tab])
                n = int(self.block_tiles[w, tab])
                first_of[s] = True
                last_of[s + n - 1] = True

        self.T = T
        self.win_of = win_of
        self.table_of = table_of
        self.first_of = first_of
        self.last_of = last_of
        self.core, self.local, self.win, self.half = core, local, win, half
        self.n_a_tiles = int(a_tiles.sum())

        # gather runs: same-table spans capped at RUN_MAX (tables are the
        # two contiguous passes, so runs only break at the pass boundary)
        runs = []
        t = 0
        while t < T:
            tab = self.table_of[t]
            e = t
            while e < T and self.table_of[e] == tab and e - t < RUN_MAX:
                e += 1
            runs.append((int(tab), t, e - t))
            t = e
        self.runs = runs

        # chunks: <=4-tile pieces within runs
        chunks = []
        for ri, (tab, t0, L) in enumerate(runs):
            t = t0
            while t < t0 + L:
                nt = min(4, t0 + L - t)
                chunks.append((ri, t, nt))
                t += nt
        self.chunks = chunks

    def key(self):
        return (
            tuple(self.win_of.tolist()),
            tuple(self.table_of.tolist()),
        )


def _pack_core(c, S, edge, idx_j):
    """Per-core padded arrays following the common schedule."""
    T = S.T
    perm = np.full(T * P, -1, dtype=np.int64)
    for w in range(W_PER_CORE):
        for tab in (0, 1):
            sel = np.flatnonzero(
                (S.core == c) & (S.win == w) & (S.half == tab)
            )
            n = len(sel)
            s0 = S.block_start[w, tab] * P
            cap = S.block_tiles[w, tab] * P
            assert n <= cap, f"schedule overflow c={c} w={w} tab={tab}"
            perm[s0:s0 + n] = sel

    valid = perm >= 0
    pidx = np.where(valid, perm, 0)

    # idx16 [128, T*8]: per tile-order flattening i=t*128+p -> [i%16, i//16]
    loc = (idx_j[pidx] - S.table_of[np.arange(T * P) // P] * HALF).astype(
        np.int64
    )
    loc[~valid] = 0
    assert (loc >= 0).all() and (loc < HALF).all()
    if os.environ.get("K_REMAP") != "0":
        # msg tables partition-major: node j at position (j%128)*TPB + j//128
        loc = (loc % P) * TPB + loc // P
    idx16 = loc.astype(np.int16).reshape(T * 8, 16).T  # [16, T*8]
    idx16 = np.tile(idx16, (8, 1)).copy()              # [128, T*8]

    # chunk-packed edge features and slots
    n_ch = len(S.chunks)
    edgeT = np.zeros((n_ch, D_EDGE, 512), dtype=DT_NP)
    slotp = np.full((n_ch, P, 4), PAD_SLOT, dtype=DT_NP)
    slots_all = (S.local[pidx] % P).astype(DT_NP)
    slots_all[~valid] = PAD_SLOT
    ev = np.zeros((T * P, D_EDGE), dtype=DT_NP)
    ev[valid] = edge[pidx[valid]]
    for k, (ri, t0, nt) in enumerate(S.chunks):
        edgeT[k, :, :nt * P] = ev[t0 * P:(t0 + nt) * P].T
        for t in range(nt):
            slotp[k, :, t] = slots_all[(t0 + t) * P:(t0 + t + 1) * P]
    return edgeT, slotp, idx16


# --------------------------------------------------------------------------
# device program
# --------------------------------------------------------------------------

def _build_program(S):
    T = S.T
    n_ch = len(S.chunks)
    dbg_skip_p1 = os.environ.get("K_SKIP_P1") == "1"
    dbg_skip_gather = os.environ.get("K_SKIP_GATHER") == "1"
    dbg_max_chunks = int(os.environ.get("K_MAX_CHUNKS", "999999"))
    dbg_stages = int(os.environ.get("K_STAGES", "99"))

    nc = bacc.Bacc(
        "TRN2", target_bir_lowering=False, debug=False, num_devices=N_CORES,
        num_swdge_queues=N_QUEUES,
    )

    # ---- I/O ----
    nodeT_h = nc.dram_tensor("nodeT", [P, N_PAD], DT, kind="ExternalInput").ap()
    Wn_h = nc.dram_tensor("Wn", [D_NODE, D_HID], DT, kind="ExternalInput").ap()
    We1_h = nc.dram_tensor("We1p", [D_EDGE, D_HID], DT, kind="ExternalInput").ap()
    We2_h = nc.dram_tensor("We2", [D_HID, D_HID], DT, kind="ExternalInput").ap()
    be1_h = nc.dram_tensor("be1c", [P, 1], F32, kind="ExternalInput").ap()
    be2_h = nc.dram_tensor("be2bc", [P, 512], F32, kind="ExternalInput").ap()
    iota_h = nc.dram_tensor("iota", [P, P], DT, kind="ExternalInput").ap()
    ones_h = nc.dram_tensor("ones1", [1, P], DT, kind="ExternalInput").ap()
    be2r_h = nc.dram_tensor("be2r", [1, 512], DT, kind="ExternalInput").ap()
    edgeT_h = nc.dram_tensor(
        "edgeT", [n_ch, D_EDGE, 512], DT, kind="ExternalInput"
    ).ap()
    slot_h = nc.dram_tensor(
        "slotp", [n_ch, P, 4], DT, kind="ExternalInput"
    ).ap()
    idx16_h = nc.dram_tensor(
        "idx16", [P, T * 8], I16, kind="ExternalInput"
    ).ap()
    out_h = nc.dram_tensor(
        "out", [W_PER_CORE * P, D_HID], F32, kind="ExternalOutput"
    ).ap()

    msg_h = [
        nc.dram_tensor("msgA", [HALF, D_HID], DT).ap(),
        nc.dram_tensor("msgB", [HALF, D_HID], DT).ap(),
    ]

    LR = mybir.ActivationFunctionType.Prelu

    with tile.TileContext(nc) as tc:
        with tc.tile_pool(name="consts", bufs=1) as cpool:
            Wn_sb = cpool.tile([D_NODE, D_HID], DT)
            nc.sync.dma_start(Wn_sb[:], Wn_h[:])
            We1_sb = cpool.tile([D_EDGE, D_HID], DT)
            nc.sync.dma_start(We1_sb[:], We1_h[:])
            We2_sb = cpool.tile([D_HID, D_HID], DT)
            nc.sync.dma_start(We2_sb[:], We2_h[:])
            be1_sb = cpool.tile([P, 1], F32)
            nc.sync.dma_start(be1_sb[:], be1_h[:])
            be2_sb = cpool.tile([P, 512], F32)
            nc.sync.dma_start(be2_sb[:], be2_h[:])
            iota_sb = cpool.tile([P, P], DT)
            nc.sync.dma_start(iota_sb[:], iota_h[:])
            ones_sb = cpool.tile([1, P], DT)
            nc.sync.dma_start(ones_sb[:], ones_h[:])
            be2r_sb = cpool.tile([1, 512], DT)
            nc.sync.dma_start(be2r_sb[:], be2r_h[:])
            # whole idx table up front (frees the sync queue during phase 2
            # and removes per-run ix waits on the gather engine)
            ix_all = cpool.tile([P, T * 8], I16)
            nc.sync.dma_start(ix_all[:], idx16_h[:])

            # ---- phase 1: msg = node @ Wn (1024-col slabs; A half first) ----
            # slab list: per table, 24 x 1024 + 1 x 512 (HALF = 25088 rows)
            slabs = []
            for tab in (0, 1):
                for k in range(24):
                    slabs.append((tab, k * 1024, 1024))
                slabs.append((tab, 24 * 1024, 512))
            p1_stores = [[], []]   # per table
            with (
                tc.tile_pool(name="p1_in", bufs=3) as p1in,
                tc.tile_pool(name="p1_stage", bufs=3) as p1st,
                tc.tile_pool(name="p1_psum", bufs=2, space="PSUM") as p1ps,
            ):
                for g, (tab, r0, ncols) in enumerate(
                    [] if dbg_skip_p1 else slabs
                ):
                    col0 = tab * HALF + r0
                    nt_sb = p1in.tile([P, 1024], DT, tag="nodeT")
                    # phase-1 loads on the gpsimd queue (idle in phase 1),
                    # stores on scalar: the sync queue stays free for phase 2
                    nc.gpsimd.dma_start(
                        nt_sb[:, :ncols], nodeT_h[:, col0:col0 + ncols]
                    )
                    ps = p1ps.tile([P, 1024], F32, tag="p1ps")
                    for t in range(ncols // P):
                        nc.tensor.matmul(
                            ps[:, t * P:(t + 1) * P],
                            lhsT=nt_sb[:, t * P:(t + 1) * P],
                            rhs=Wn_sb[:],
                            start=True,
                            stop=True,
                        )
                    stage = p1st.tile([P, 1024], DT, tag="p1stage")
                    if g % 2 == 0:
                        nc.vector.tensor_copy(stage[:, :ncols], ps[:, :ncols])
                    else:
                        nc.scalar.activation(
                            stage[:, :ncols], ps[:, :ncols],
                            mybir.ActivationFunctionType.Copy,
                        )
                    # table is partition-major: slab rows j=r0+t*128+p land at
                    # k = p*TPB + (r0//128 + t) -> contiguous 2KB per partition
                    if os.environ.get("K_REMAP") != "0":
                        t0 = r0 // P
                        dst = msg_h[tab][:].rearrange(
                            "(p t) f -> p t f", t=TPB
                        )[:, t0:t0 + ncols // P, :]
                    else:
                        dst = msg_h[tab][r0:r0 + ncols, :].rearrange(
                            "(t p) f -> p t f", p=P
                        )
                    srcap = stage[:, :ncols].rearrange(
                        "p (t f) -> p t f", t=ncols // P
                    )
                    st_inst = nc.scalar.dma_start(dst, srcap)
                    p1_stores[tab].append(st_inst.ins)

            # ---- phase 2: pass A then pass B ----
            with (
                tc.tile_pool(name="p2_in", bufs=6) as p2in,
                tc.tile_pool(name="p2_g", bufs=8) as p2g,
                tc.tile_pool(name="p2_mid", bufs=3) as p2mid,
                tc.tile_pool(name="p2_acc", bufs=1) as accp,
                tc.tile_pool(name="h1_psum", bufs=2, space="PSUM") as h1ps,
                tc.tile_pool(name="h2_psum", bufs=2, space="PSUM") as h2ps,
                tc.tile_pool(name="out_psum", bufs=2, space="PSUM") as outps,
                tc.tile_pool(name="out_stage", bufs=3) as outst,
            ):
                cur_out = {}
                acc = {}

                chunks_by_run = {}
                for k, (ri, t0, nt) in enumerate(S.chunks):
                    chunks_by_run.setdefault(ri, []).append((k, t0, nt))

                first_run_of_tab = {}
                for ri, (tab, rt0, L) in enumerate(S.runs):
                    if tab not in first_run_of_tab:
                        first_run_of_tab[tab] = ri

                for ri, (tab, rt0, L) in enumerate(S.runs):
                    G = p2g.tile(
                        [P, RUN_MAX * P], DT, tag="G", name=f"G_r{ri}"
                    )
                    if dbg_skip_gather:
                        nc.gpsimd.memset(G[:, :L * P], 0.5)
                    else:
                        g_inst = nc.gpsimd.dma_gather(
                            G[:, :L * P].rearrange("p (g f) -> p g f", f=P),
                            msg_h[tab][:],
                            ix_all[:, rt0 * 8:(rt0 + L) * 8],
                            num_idxs=L * P,
                            num_idxs_reg=L * P,
                            elem_size=P,
                            elem_step=P,
                            queue_num=ri % N_QUEUES,
                        )
                        if ri == first_run_of_tab.get(tab):
                            for st in p1_stores[tab]:
                                add_dep_helper(
                                    g_inst.ins, st, sync=True,
                                    reason=f"gather after msg{tab} stores",
                                )

                    # one edgeT/slot DMA covers the whole run's chunks
                    rchunks = [c for c in chunks_by_run[ri]
                               if c[0] < dbg_max_chunks]
                    if rchunks:
                        k0 = rchunks[0][0]
                        nk = len(rchunks)
                        et_run = p2in.tile(
                            [D_EDGE, 2 * 512], DT, tag="edgeT"
                        )
                        nc.sync.dma_start(
                            et_run[:, :nk * 512].rearrange(
                                "e (k c) -> e k c", k=nk
                            ),
                            edgeT_h[k0:k0 + nk, :, :].rearrange(
                                "k e c -> e k c"
                            ),
                        )
                        sl_run = p2in.tile([P, 2 * 4], DT, tag="slot")
                        nc.sync.dma_start(
                            sl_run[:, :nk * 4].rearrange(
                                "p (k c) -> p k c", k=nk
                            ),
                            slot_h[k0:k0 + nk, :, :].rearrange(
                                "k p c -> p k c"
                            ),
                        )

                    for (k, t0, nt) in rchunks:
                        ncols = nt * P
                        goff = (t0 - rt0) * P
                        kk = k - rchunks[0][0]
                        et_sb = et_run[:, kk * 512:kk * 512 + 512]
                        sl_sb = sl_run[:, kk * 4:kk * 4 + 4]

                        # h1 = lrelu(edge @ We1 + be1), feature-major [h x e]
                        ps1 = h1ps.tile([P, 512], F32, tag="h1ps")
                        nc.tensor.matmul(
                            ps1[:, :ncols],
                            lhsT=We1_sb[:],
                            rhs=et_sb[:, :ncols],
                            start=True,
                            stop=True,
                        )
                        h1f = p2mid.tile([P, 512], DT, tag="h1f")
                        if dbg_stages >= 2:
                            nc.scalar.activation(
                                h1f[:, :ncols], ps1[:, :ncols], LR,
                                bias=be1_sb[:], scale=1.0, alpha=NEG_SLOPE,
                            )
                        else:
                            nc.vector.tensor_copy(h1f[:, :ncols], ps1[:, :ncols])
                        if dbg_stages < 3:
                            continue

                        # h2 = h1.T @ We2 + be2, edge-major [e x h];
                        # the bias lands via a K=1 accumulate matmul (PE, not DVE)
                        ps2 = h2ps.tile([P, 512], F32, tag="h2ps")
                        use_bias_mm = os.environ.get("K_BIAS_MM") == "1"
                        mm2s = []
                        for t in range(nt):
                            mm2s.append(nc.tensor.matmul(
                                ps2[:, t * P:(t + 1) * P],
                                lhsT=h1f[:, t * P:(t + 1) * P],
                                rhs=We2_sb[:],
                                start=True,
                                stop=not use_bias_mm,
                            ))
                        if use_bias_mm:
                            bmm = nc.tensor.matmul(
                                ps2[:, :ncols],
                                lhsT=ones_sb[:],
                                rhs=be2r_sb[:, :ncols],
                                start=False,
                                stop=True,
                            )
                            for m in mm2s:
                                add_dep_helper(
                                    bmm.ins, m.ins, sync=False,
                                    reason="bias accumulate after mm2 blocks",
                                )
                        else:
                            nc.vector.tensor_tensor(
                                ps2[:, :ncols], in0=ps2[:, :ncols],
                                in1=be2_sb[:, :ncols], op=mybir.AluOpType.add,
                            )
                        eh = p2mid.tile([P, 512], DT, tag="eh")
                        if dbg_stages >= 4:
                            nc.scalar.activation(
                                eh[:, :ncols], ps2[:, :ncols], LR,
                                scale=1.0, alpha=NEG_SLOPE,
                            )
                        else:
                            nc.vector.tensor_copy(eh[:, :ncols], ps2[:, :ncols])
                        if dbg_stages < 5:
                            continue

                        # onehot[e, s] = (slot[e] == s), all nt tiles in one op
                        oh = p2mid.tile([P, 512], DT, tag="oh")
                        if dbg_stages >= 5:
                            nc.vector.tensor_tensor(
                                oh[:, :ncols].rearrange(
                                    "p (t f) -> p t f", t=nt
                                ),
                                in0=sl_sb[:, :nt].rearrange(
                                    "p (t o) -> p t o", o=1
                                ).to_broadcast([P, nt, P]),
                                in1=iota_sb[:].rearrange(
                                    "p (o f) -> p o f", o=1
                                ).to_broadcast([P, nt, P]),
                                op=mybir.AluOpType.is_equal,
                            )
                        else:
                            nc.gpsimd.memset(oh[:, :ncols], 0.0)
                        if dbg_stages < 6:
                            continue

                        # product = gathered msg * edge_h
                        pr = p2mid.tile([P, 512], DT, tag="pr")
                        nc.vector.tensor_tensor(
                            pr[:, :ncols],
                            in0=G[:, goff:goff + ncols],
                            in1=eh[:, :ncols],
                            op=mybir.AluOpType.mult,
                        )

                        if dbg_stages < 7:
                            continue
                        # scatter: out_w[s, f] += onehot[:, t].T @ product[:, t]
                        for t in range(nt):
                            i = t0 + t
                            w = int(S.win_of[i])
                            if S.first_of[i]:
                                cur_out[w] = outps.tile(
                                    [P, P], F32, tag="outp",
                                    name=f"outp_w{w}t{tab}"
                                )
                            nc.tensor.matmul(
                                cur_out[w][:],
                                lhsT=oh[:, t * P:(t + 1) * P],
                                rhs=pr[:, t * P:(t + 1) * P],
                                start=bool(S.first_of[i]),
                                stop=bool(S.last_of[i]),
                            )
                            if S.last_of[i]:
                                if tab == 0:
                                    # pass A: stash partial in SBUF (scalar
                                    # engine copy; DVE is the busier one)
                                    a = accp.tile(
                                        [P, P], F32, tag=f"acc_w{w}",
                                        name=f"acc_w{w}"
                                    )
                                    nc.scalar.activation(
                                        a[:], cur_out[w][:],
                                        mybir.ActivationFunctionType.Copy,
                                    )
                                    acc[w] = a
                                else:
                                    # pass B: add pass-A partial, store out
                                    st = outst.tile(
                                        [P, P], F32, tag="outstage",
                                        name=f"outst_w{w}"
                                    )
                                    nc.vector.tensor_tensor(
                                        st[:], in0=cur_out[w][:],
                                        in1=acc[w][:],
                                        op=mybir.AluOpType.add,
                                    )
                                    nc.sync.dma_start(
                                        out_h[w * P:(w + 1) * P, :], st[:]
                                    )
                                del cur_out[w]

    nc.compile()
    return nc


# --------------------------------------------------------------------------
# entry point
# --------------------------------------------------------------------------

def kernel(node, edge, Wn, We1, be1, We2, be2, seg_i, idx_j):
    global LAST_RESULT
    node = np.asarray(node, dtype=np.float32)
    edge = np.asarray(edge, dtype=np.float32)
    Wn = np.asarray(Wn, dtype=np.float32)
    We1 = np.asarray(We1, dtype=np.float32)
    be1 = np.asarray(be1, dtype=np.float32)
    We2 = np.asarray(We2, dtype=np.float32)
    be2 = np.asarray(be2, dtype=np.float32)
    seg_i = np.asarray(seg_i, dtype=np.int32)
    idx_j = np.asarray(idx_j, dtype=np.int32)

    S = Schedule(seg_i.astype(np.int64), idx_j.astype(np.int64))
    key = S.key()
    if key not in _PROGRAM_CACHE:
        _PROGRAM_CACHE[key] = _build_program(S)
    nc = _PROGRAM_CACHE[key]

    nodeT = np.zeros((P, N_PAD), dtype=DT_NP)
    nodeT[:, :N_NODES] = node.T
    iota = np.broadcast_to(np.arange(P, dtype=DT_NP), (P, P)).copy()
    common = {
        "nodeT": nodeT,
        "Wn": Wn.astype(DT_NP),
        "We1p": We1.astype(DT_NP),
        "We2": We2.astype(DT_NP),
        "be1c": be1.reshape(P, 1).copy(),
        "be2bc": np.broadcast_to(
            np.tile(be2, 4), (P, 512)
        ).astype(np.float32).copy(),
        "iota": iota,
        "ones1": np.ones((1, P), dtype=DT_NP),
        "be2r": np.tile(be2, 4).reshape(1, 512).astype(DT_NP),
    }
    in_maps = []
    for c in range(N_CORES):
        edgeT, slotp, idx16 = _pack_core(c, S, edge, idx_j)
        m = dict(common)
        m["edgeT"] = edgeT
        m["slotp"] = slotp
        m["idx16"] = idx16
        in_maps.append(m)

    if TRACE:
        _ensure_ntff_hook()
    res = run_bass_kernel_spmd(
        nc, in_maps, list(range(N_CORES)), trace=TRACE
    )
    LAST_RESULT = res
    out = np.concatenate(
        [res.results[c]["out"][:NPC] for c in range(N_CORES)], axis=0
    )
    return out.astype(np.float32)


# revision 33
# speedup vs baseline: 2.0482x; 1.0496x over previous
"""Trainium2 Bass kernel for GNN message passing.

Computes, for full inputs:
    edge_h = lrelu(lrelu(edge @ We1 + be1) @ We2 + be2)        # [E, 128]
    out    = segment_sum((node @ Wn)[idx_j] * edge_h, seg_i)   # [N, 128]

Strategy (8 NeuronCores, SPMD single program):
  - Shard edges by DESTINATION range (N/8 = 6250 nodes per core). Each core
    produces its own output slice -> no collectives; host concatenates.
  - Phase 1 (replicated on every core): msg = node @ Wn written to two
    internal DRAM tables (halves, so gather indices fit int16), row-major
    fp16 (256B rows -> full-rate gather). msgA slabs are written first so
    pass-A gathers can start while msgB is still being computed.
  - Phase 2, two passes: pass A processes every window's msgA-sourced edges
    (gathers read only msgA), pass B the msgB-sourced ones. Per 128-edge
    tile: edge MLP via PE matmuls, LeakyReLU (Prelu) on the scalar engine,
    one-hot(slot) on DVE, scatter-add = one-hot matmul accumulated in a
    PSUM window [128 slots x 128 feat]. Pass-A windows flush to SBUF
    accumulators; pass-B flush adds the accumulator and stores to DRAM.
  - Gathers (InstDMAGatherAnt) are issued round-robin on 4 SWDGE queues
    (independent Q7 core pairs) with 6 G buffers, so up to 4 descriptor
    generations overlap: ~3-4 ns/idx instead of ~9 ns/idx single-queue.

The tile->window/table schedule is data-dependent and baked into the
program at build time (all cores share it; per-core data is padded to the
common schedule). Programs are cached per schedule within the process.
"""

import os
import sys
import types

import numpy as np

import concourse.bass as bass
import concourse.tile as tile
from concourse.tile import add_dep_helper
from concourse import bacc, mybir
from concourse.bass_utils import run_bass_kernel_spmd

# ---- problem constants (hardcoded per spec) ----
N_NODES = 50000
D_NODE = 128
D_EDGE = 32
D_HID = 128
N_CORES = 8
NPC = N_NODES // N_CORES          # nodes per core = 6250
P = 128                           # partitions
W_PER_CORE = (NPC + P - 1) // P   # 49 windows per core
NEG_SLOPE = 0.01
PAD_SLOT = 300.0                  # slot value that never matches iota 0..127

# node rows padded to a multiple of 1024 so the two msg halves split evenly
N_PAD = ((N_NODES + 1023) // 1024) * 1024    # 50176
N_SLABS = N_PAD // 512                       # 98
HALF = N_PAD // 2                            # 25088 rows per msg table
HALF_SLABS = N_SLABS // 2                    # 49

RUN_MAX = 8          # max tiles per dma_gather (1024 idxs; ring cap, >1920 wedges)
N_QUEUES = 4         # SWDGE queues (Q7 core pairs) for parallel desc-gen
TPB = HALF // P      # 196: msg table stored partition-major, k = p*TPB + t

F32 = mybir.dt.float32
F16 = mybir.dt.float16
I16 = mybir.dt.int16
# data dtype for matmul operands / msg tables (fp16 default; K_DT=f32 to revert)
DT = F32 if os.environ.get("K_DT") == "f32" else F16
DT_NP = np.float32 if os.environ.get("K_DT") == "f32" else np.float16

TRACE = False
LAST_RESULT = None

_PROGRAM_CACHE = {}


def _ensure_ntff_hook():
    """Provide antenv.axon_hooks if this image's antenv lacks it, and
    register the ctypes NTFF profiling hook so trace=True works."""
    try:
        from antenv.axon_hooks import get_axon_ntff_profile_hook  # noqa: F401
        return
    except ImportError:
        pass
    mod = types.ModuleType("antenv.axon_hooks")
    _hook = [None]
    mod.set_axon_ntff_profile_hook = lambda h: _hook.__setitem__(0, h)
    mod.get_axon_ntff_profile_hook = lambda: _hook[0]
    sys.modules["antenv.axon_hooks"] = mod
    import antenv

    antenv.axon_hooks = mod
    try:
        from trn_agent_boot.trn_boot import _ntff_profile_via_ctypes

        mod.set_axon_ntff_profile_hook(
            _ntff_profile_via_ctypes("/opt/axon/libaxon_pjrt.so")
        )
    except Exception:
        pass


# --------------------------------------------------------------------------
# host-side schedule + packing
# --------------------------------------------------------------------------

def cdiv(a, b):
    return (a + b - 1) // b


class Schedule:
    """Common (all-core) static schedule baked into the program.

    Tile sequence = pass A (all windows, table 0) then pass B (table 1).
    """

    def __init__(self, seg_i, idx_j):
        core = seg_i // NPC
        local = seg_i - core * NPC
        win = local // P
        half = (idx_j >= HALF).astype(np.int64)

        cnt = np.zeros((2, N_CORES, W_PER_CORE), dtype=np.int64)
        for c in range(N_CORES):
            m = core == c
            for h in (0, 1):
                cnt[h, c] = np.bincount(
                    win[m & (half == h)], minlength=W_PER_CORE
                )
        a_tiles = np.maximum(cdiv(cnt[0].max(axis=0), P), 1)
        b_tiles = np.maximum(cdiv(cnt[1].max(axis=0), P), 1)

        # tile sequence: pass A (tab 0) then pass B (tab 1)
        win_of, table_of = [], []
        self.block_start = np.zeros((W_PER_CORE, 2), dtype=np.int64)
        self.block_tiles = np.zeros((W_PER_CORE, 2), dtype=np.int64)
        for tab in (0, 1):
            nts = a_tiles if tab == 0 else b_tiles
            for w in range(W_PER_CORE):
                self.block_start[w, tab] = len(win_of)
                self.block_tiles[w, tab] = nts[w]
                for _ in range(int(nts[w])):
                    win_of.append(w)
                    table_of.append(tab)
        win_of = np.array(win_of, dtype=np.int64)
        table_of = np.array(table_of, dtype=np.int64)
        T = len(win_of)
        # first/last per (w, tab) block
        first_of = np.zeros(T, dtype=bool)
        last_of = np.zeros(T, dtype=bool)
        for w in range(W_PER_CORE):
            for tab in (0, 1):
                s = int(self.block_start[w, tab])
                n = int(self.block_tiles[w, tab])
                first_of[s] = True
                last_of[s + n - 1] = True

        self.T = T
        self.win_of = win_of
        self.table_of = table_of
        self.first_of = first_of
        self.last_of = last_of
        self.core, self.local, self.win, self.half = core, local, win, half
        self.n_a_tiles = int(a_tiles.sum())

        # gather runs: same-table spans capped at RUN_MAX (tables are the
        # two contiguous passes, so runs only break at the pass boundary)
        runs = []
        t = 0
        while t < T:
            tab = self.table_of[t]
            e = t
            while e < T and self.table_of[e] == tab and e - t < RUN_MAX:
                e += 1
            runs.append((int(tab), t, e - t))
            t = e
        self.runs = runs

        # chunks: <=4-tile pieces within runs
        chunks = []
        for ri, (tab, t0, L) in enumerate(runs):
            t = t0
            while t < t0 + L:
                nt = min(4, t0 + L - t)
                chunks.append((ri, t, nt))
                t += nt
        self.chunks = chunks

    def key(self):
        return (
            tuple(self.win_of.tolist()),
            tuple(self.table_of.tolist()),
        )


def _pack_core(c, S, edge, idx_j):
    """Per-core padded arrays following the common schedule."""
    T = S.T
    perm = np.full(T * P, -1, dtype=np.int64)
    for w in range(W_PER_CORE):
        for tab in (0, 1):
            sel = np.flatnonzero(
                (S.core == c) & (S.win == w) & (S.half == tab)
            )
            n = len(sel)
            s0 = S.block_start[w, tab] * P
            cap = S.block_tiles[w, tab] * P
            assert n <= cap, f"schedule overflow c={c} w={w} tab={tab}"
            perm[s0:s0 + n] = sel

    valid = perm >= 0
    pidx = np.where(valid, perm, 0)

    # idx16 [128, T*8]: per tile-order flattening i=t*128+p -> [i%16, i//16]
    loc = (idx_j[pidx] - S.table_of[np.arange(T * P) // P] * HALF).astype(
        np.int64
    )
    loc[~valid] = 0
    assert (loc >= 0).all() and (loc < HALF).all()
    if os.environ.get("K_REMAP") != "0":
        # msg tables partition-major: node j at position (j%128)*TPB + j//128
        loc = (loc % P) * TPB + loc // P
    idx16 = loc.astype(np.int16).reshape(T * 8, 16).T  # [16, T*8]
    idx16 = np.tile(idx16, (8, 1)).copy()              # [128, T*8]

    # chunk-packed edge features and slots
    n_ch = len(S.chunks)
    edgeT = np.zeros((n_ch, D_EDGE, 512), dtype=DT_NP)
    slotp = np.full((n_ch, P, 4), PAD_SLOT, dtype=DT_NP)
    slots_all = (S.local[pidx] % P).astype(DT_NP)
    slots_all[~valid] = PAD_SLOT
    ev = np.zeros((T * P, D_EDGE), dtype=DT_NP)
    ev[valid] = edge[pidx[valid]]
    for k, (ri, t0, nt) in enumerate(S.chunks):
        edgeT[k, :, :nt * P] = ev[t0 * P:(t0 + nt) * P].T
        for t in range(nt):
            slotp[k, :, t] = slots_all[(t0 + t) * P:(t0 + t + 1) * P]
    return edgeT, slotp, idx16


# --------------------------------------------------------------------------
# device program
# --------------------------------------------------------------------------

def _build_program(S):
    T = S.T
    n_ch = len(S.chunks)
    dbg_skip_p1 = os.environ.get("K_SKIP_P1") == "1"
    dbg_skip_gather = os.environ.get("K_SKIP_GATHER") == "1"
    dbg_max_chunks = int(os.environ.get("K_MAX_CHUNKS", "999999"))
    dbg_stages = int(os.environ.get("K_STAGES", "99"))

    nc = bacc.Bacc(
        "TRN2", target_bir_lowering=False, debug=False, num_devices=N_CORES,
        num_swdge_queues=N_QUEUES,
    )

    # ---- I/O ----
    nodeT_h = nc.dram_tensor("nodeT", [P, N_PAD], DT, kind="ExternalInput").ap()
    Wn_h = nc.dram_tensor("Wn", [D_NODE, D_HID], DT, kind="ExternalInput").ap()
    We1_h = nc.dram_tensor("We1p", [D_EDGE, D_HID], DT, kind="ExternalInput").ap()
    We2_h = nc.dram_tensor("We2", [D_HID, D_HID], DT, kind="ExternalInput").ap()
    be1_h = nc.dram_tensor("be1c", [P, 1], F32, kind="ExternalInput").ap()
    be2_h = nc.dram_tensor("be2bc", [P, 512], F32, kind="ExternalInput").ap()
    iota_h = nc.dram_tensor("iota", [P, P], DT, kind="ExternalInput").ap()
    ones_h = nc.dram_tensor("ones1", [1, P], DT, kind="ExternalInput").ap()
    be2r_h = nc.dram_tensor("be2r", [1, 512], DT, kind="ExternalInput").ap()
    edgeT_h = nc.dram_tensor(
        "edgeT", [n_ch, D_EDGE, 512], DT, kind="ExternalInput"
    ).ap()
    slot_h = nc.dram_tensor(
        "slotp", [n_ch, P, 4], DT, kind="ExternalInput"
    ).ap()
    idx16_h = nc.dram_tensor(
        "idx16", [P, T * 8], I16, kind="ExternalInput"
    ).ap()
    out_h = nc.dram_tensor(
        "out", [W_PER_CORE * P, D_HID], F32, kind="ExternalOutput"
    ).ap()

    msg_h = [
        nc.dram_tensor("msgA", [HALF, D_HID], DT).ap(),
        nc.dram_tensor("msgB", [HALF, D_HID], DT).ap(),
    ]

    LR = mybir.ActivationFunctionType.Prelu

    with tile.TileContext(nc) as tc:
        with tc.tile_pool(name="consts", bufs=1) as cpool:
            Wn_sb = cpool.tile([D_NODE, D_HID], DT)
            nc.sync.dma_start(Wn_sb[:], Wn_h[:])
            We1_sb = cpool.tile([D_EDGE, D_HID], DT)
            nc.sync.dma_start(We1_sb[:], We1_h[:])
            We2_sb = cpool.tile([D_HID, D_HID], DT)
            nc.sync.dma_start(We2_sb[:], We2_h[:])
            be1_sb = cpool.tile([P, 1], F32)
            nc.sync.dma_start(be1_sb[:], be1_h[:])
            be2_sb = cpool.tile([P, 512], F32)
            nc.sync.dma_start(be2_sb[:], be2_h[:])
            iota_sb = cpool.tile([P, P], DT)
            nc.sync.dma_start(iota_sb[:], iota_h[:])
            ones_sb = cpool.tile([1, P], DT)
            nc.sync.dma_start(ones_sb[:], ones_h[:])
            be2r_sb = cpool.tile([1, 512], DT)
            nc.sync.dma_start(be2r_sb[:], be2r_h[:])
            # whole idx table up front (frees the sync queue during phase 2
            # and removes per-run ix waits on the gather engine)
            ix_all = cpool.tile([P, T * 8], I16)
            nc.sync.dma_start(ix_all[:], idx16_h[:])

            # ---- phase 1: msg = node @ Wn (1024-col slabs; A half first) ----
            # slab list: per table, 24 x 1024 + 1 x 512 (HALF = 25088 rows)
            slabs = []
            for tab in (0, 1):
                for k in range(24):
                    slabs.append((tab, k * 1024, 1024))
                slabs.append((tab, 24 * 1024, 512))
            p1_stores = [[], []]   # per table
            with (
                tc.tile_pool(name="p1_in", bufs=4) as p1in,
                tc.tile_pool(name="p1_stage", bufs=4) as p1st,
                tc.tile_pool(name="p1_psum", bufs=3, space="PSUM") as p1ps,
            ):
                for g, (tab, r0, ncols) in enumerate(
                    [] if dbg_skip_p1 else slabs
                ):
                    col0 = tab * HALF + r0
                    nt_sb = p1in.tile([P, 1024], DT, tag="nodeT")
                    # phase-1 loads on the gpsimd queue (idle in phase 1),
                    # stores on scalar: the sync queue stays free for phase 2
                    nc.gpsimd.dma_start(
                        nt_sb[:, :ncols], nodeT_h[:, col0:col0 + ncols]
                    )
                    ps = p1ps.tile([P, 1024], F32, tag="p1ps")
                    for t in range(ncols // P):
                        nc.tensor.matmul(
                            ps[:, t * P:(t + 1) * P],
                            lhsT=nt_sb[:, t * P:(t + 1) * P],
                            rhs=Wn_sb[:],
                            start=True,
                            stop=True,
                        )
                    stage = p1st.tile([P, 1024], DT, tag="p1stage")
                    if g % 2 == 0:
                        nc.vector.tensor_copy(stage[:, :ncols], ps[:, :ncols])
                    else:
                        nc.scalar.activation(
                            stage[:, :ncols], ps[:, :ncols],
                            mybir.ActivationFunctionType.Copy,
                        )
                    # table is partition-major: slab rows j=r0+t*128+p land at
                    # k = p*TPB + (r0//128 + t) -> contiguous 2KB per partition
                    if os.environ.get("K_REMAP") != "0":
                        t0 = r0 // P
                        dst = msg_h[tab][:].rearrange(
                            "(p t) f -> p t f", t=TPB
                        )[:, t0:t0 + ncols // P, :]
                    else:
                        dst = msg_h[tab][r0:r0 + ncols, :].rearrange(
                            "(t p) f -> p t f", p=P
                        )
                    srcap = stage[:, :ncols].rearrange(
                        "p (t f) -> p t f", t=ncols // P
                    )
                    st_inst = nc.scalar.dma_start(dst, srcap)
                    p1_stores[tab].append(st_inst.ins)

            # ---- phase 2: pass A then pass B ----
            with (
                tc.tile_pool(name="p2_in", bufs=6) as p2in,
                tc.tile_pool(name="p2_g", bufs=12) as p2g,
                tc.tile_pool(name="p2_mid", bufs=3) as p2mid,
                tc.tile_pool(name="p2_acc", bufs=1) as accp,
                tc.tile_pool(name="h1_psum", bufs=2, space="PSUM") as h1ps,
                tc.tile_pool(name="h2_psum", bufs=2, space="PSUM") as h2ps,
                tc.tile_pool(name="out_psum", bufs=2, space="PSUM") as outps,
                tc.tile_pool(name="out_stage", bufs=3) as outst,
            ):
                cur_out = {}
                acc = {}
                reg_full = nc.gpsimd.to_reg(RUN_MAX * P)

                chunks_by_run = {}
                for k, (ri, t0, nt) in enumerate(S.chunks):
                    chunks_by_run.setdefault(ri, []).append((k, t0, nt))

                first_run_of_tab = {}
                for ri, (tab, rt0, L) in enumerate(S.runs):
                    if tab not in first_run_of_tab:
                        first_run_of_tab[tab] = ri

                for ri, (tab, rt0, L) in enumerate(S.runs):
                    G = p2g.tile(
                        [P, RUN_MAX * P], DT, tag="G", name=f"G_r{ri}"
                    )
                    if dbg_skip_gather:
                        nc.gpsimd.memset(G[:, :L * P], 0.5)
                    else:
                        g_inst = nc.gpsimd.dma_gather(
                            G[:, :L * P].rearrange("p (g f) -> p g f", f=P),
                            msg_h[tab][:],
                            ix_all[:, rt0 * 8:(rt0 + L) * 8],
                            num_idxs=L * P,
                            num_idxs_reg=(
                                reg_full if L == RUN_MAX else L * P
                            ),
                            elem_size=P,
                            elem_step=P,
                            queue_num=ri % N_QUEUES,
                        )
                        if ri == first_run_of_tab.get(tab):
                            for st in p1_stores[tab]:
                                add_dep_helper(
                                    g_inst.ins, st, sync=True,
                                    reason=f"gather after msg{tab} stores",
                                )

                    # one edgeT/slot DMA covers the whole run's chunks
                    rchunks = [c for c in chunks_by_run[ri]
                               if c[0] < dbg_max_chunks]
                    if rchunks:
                        k0 = rchunks[0][0]
                        nk = len(rchunks)
                        et_run = p2in.tile(
                            [D_EDGE, 2 * 512], DT, tag="edgeT"
                        )
                        nc.sync.dma_start(
                            et_run[:, :nk * 512].rearrange(
                                "e (k c) -> e k c", k=nk
                            ),
                            edgeT_h[k0:k0 + nk, :, :].rearrange(
                                "k e c -> e k c"
                            ),
                        )
                        sl_run = p2in.tile([P, 2 * 4], DT, tag="slot")
                        nc.sync.dma_start(
                            sl_run[:, :nk * 4].rearrange(
                                "p (k c) -> p k c", k=nk
                            ),
                            slot_h[k0:k0 + nk, :, :].rearrange(
                                "k p c -> p k c"
                            ),
                        )

                    for (k, t0, nt) in rchunks:
                        ncols = nt * P
                        goff = (t0 - rt0) * P
                        kk = k - rchunks[0][0]
                        et_sb = et_run[:, kk * 512:kk * 512 + 512]
                        sl_sb = sl_run[:, kk * 4:kk * 4 + 4]

                        # h1 = lrelu(edge @ We1 + be1), feature-major [h x e]
                        ps1 = h1ps.tile([P, 512], F32, tag="h1ps")
                        nc.tensor.matmul(
                            ps1[:, :ncols],
                            lhsT=We1_sb[:],
                            rhs=et_sb[:, :ncols],
                            start=True,
                            stop=True,
                        )
                        h1f = p2mid.tile([P, 512], DT, tag="h1f")
                        if dbg_stages >= 2:
                            nc.scalar.activation(
                                h1f[:, :ncols], ps1[:, :ncols], LR,
                                bias=be1_sb[:], scale=1.0, alpha=NEG_SLOPE,
                            )
                        else:
                            nc.vector.tensor_copy(h1f[:, :ncols], ps1[:, :ncols])
                        if dbg_stages < 3:
                            continue

                        # h2 = h1.T @ We2 + be2, edge-major [e x h];
                        # the bias lands via a K=1 accumulate matmul (PE, not DVE)
                        ps2 = h2ps.tile([P, 512], F32, tag="h2ps")
                        use_bias_mm = os.environ.get("K_BIAS_MM") == "1"
                        mm2s = []
                        for t in range(nt):
                            mm2s.append(nc.tensor.matmul(
                                ps2[:, t * P:(t + 1) * P],
                                lhsT=h1f[:, t * P:(t + 1) * P],
                                rhs=We2_sb[:],
                                start=True,
                                stop=not use_bias_mm,
                            ))
                        if use_bias_mm:
                            bmm = nc.tensor.matmul(
                                ps2[:, :ncols],
                                lhsT=ones_sb[:],
                                rhs=be2r_sb[:, :ncols],
                                start=False,
                                stop=True,
                            )
                            for m in mm2s:
                                add_dep_helper(
                                    bmm.ins, m.ins, sync=False,
                                    reason="bias accumulate after mm2 blocks",
                                )
                        else:
                            nc.vector.tensor_tensor(
                                ps2[:, :ncols], in0=ps2[:, :ncols],
                                in1=be2_sb[:, :ncols], op=mybir.AluOpType.add,
                            )
                        eh = p2mid.tile([P, 512], DT, tag="eh")
                        if dbg_stages >= 4:
                            nc.scalar.activation(
                                eh[:, :ncols], ps2[:, :ncols], LR,
                                scale=1.0, alpha=NEG_SLOPE,
                            )
                        else:
                            nc.vector.tensor_copy(eh[:, :ncols], ps2[:, :ncols])
                        if dbg_stages < 5:
                            continue

                        # onehot[e, s] = (slot[e] == s), all nt tiles in one op
                        oh = p2mid.tile([P, 512], DT, tag="oh")
                        if dbg_stages >= 5:
                            nc.vector.tensor_tensor(
                                oh[:, :ncols].rearrange(
                                    "p (t f) -> p t f", t=nt
                                ),
                                in0=sl_sb[:, :nt].rearrange(
                                    "p (t o) -> p t o", o=1
                                ).to_broadcast([P, nt, P]),
                                in1=iota_sb[:].rearrange(
                                    "p (o f) -> p o f", o=1
                                ).to_broadcast([P, nt, P]),
                                op=mybir.AluOpType.is_equal,
                            )
                        else:
                            nc.gpsimd.memset(oh[:, :ncols], 0.0)
                        if dbg_stages < 6:
                            continue

                        # product = gathered msg * edge_h
                        pr = p2mid.tile([P, 512], DT, tag="pr")
                        nc.vector.tensor_tensor(
                            pr[:, :ncols],
                            in0=G[:, goff:goff + ncols],
                            in1=eh[:, :ncols],
                            op=mybir.AluOpType.mult,
                        )

                        if dbg_stages < 7:
                            continue
                        # scatter: out_w[s, f] += onehot[:, t].T @ product[:, t]
                        for t in range(nt):
                            i = t0 + t
                            w = int(S.win_of[i])
                            if S.first_of[i]:
                                cur_out[w] = outps.tile(
                                    [P, P], F32, tag="outp",
                                    name=f"outp_w{w}t{tab}"
                                )
                            nc.tensor.matmul(
                                cur_out[w][:],
                                lhsT=oh[:, t * P:(t + 1) * P],
                                rhs=pr[:, t * P:(t + 1) * P],
                                start=bool(S.first_of[i]),
                                stop=bool(S.last_of[i]),
                            )
                            if S.last_of[i]:
                                if tab == 0:
                                    # pass A: stash partial in SBUF (scalar
                                    # engine copy; DVE is the busier one)
                                    a = accp.tile(
                                        [P, P], F32, tag=f"acc_w{w}",
                                        name=f"acc_w{w}"
                                    )
                                    nc.scalar.activation(
                                        a[:], cur_out[w][:],
                                        mybir.ActivationFunctionType.Copy,
                                    )
                                    acc[w] = a
                                else:
                                    # pass B: add pass-A partial, store out
                                    st = outst.tile(
                                        [P, P], F32, tag="outstage",
                                        name=f"outst_w{w}"
                                    )
                                    nc.vector.tensor_tensor(
                                        st[:], in0=cur_out[w][:],
                                        in1=acc[w][:],
                                        op=mybir.AluOpType.add,
                                    )
                                    nc.sync.dma_start(
                                        out_h[w * P:(w + 1) * P, :], st[:]
                                    )
                                del cur_out[w]

    nc.compile()
    return nc


# --------------------------------------------------------------------------
# entry point
# --------------------------------------------------------------------------

def kernel(node, edge, Wn, We1, be1, We2, be2, seg_i, idx_j):
    global LAST_RESULT
    node = np.asarray(node, dtype=np.float32)
    edge = np.asarray(edge, dtype=np.float32)
    Wn = np.asarray(Wn, dtype=np.float32)
    We1 = np.asarray(We1, dtype=np.float32)
    be1 = np.asarray(be1, dtype=np.float32)
    We2 = np.asarray(We2, dtype=np.float32)
    be2 = np.asarray(be2, dtype=np.float32)
    seg_i = np.asarray(seg_i, dtype=np.int32)
    idx_j = np.asarray(idx_j, dtype=np.int32)

    S = Schedule(seg_i.astype(np.int64), idx_j.astype(np.int64))
    key = S.key()
    if key not in _PROGRAM_CACHE:
        _PROGRAM_CACHE[key] = _build_program(S)
    nc = _PROGRAM_CACHE[key]

    nodeT = np.zeros((P, N_PAD), dtype=DT_NP)
    nodeT[:, :N_NODES] = node.T
    iota = np.broadcast_to(np.arange(P, dtype=DT_NP), (P, P)).copy()
    common = {
        "nodeT": nodeT,
        "Wn": Wn.astype(DT_NP),
        "We1p": We1.astype(DT_NP),
        "We2": We2.astype(DT_NP),
        "be1c": be1.reshape(P, 1).copy(),
        "be2bc": np.broadcast_to(
            np.tile(be2, 4), (P, 512)
        ).astype(np.float32).copy(),
        "iota": iota,
        "ones1": np.ones((1, P), dtype=DT_NP),
        "be2r": np.tile(be2, 4).reshape(1, 512).astype(DT_NP),
    }
    in_maps = []
    for c in range(N_CORES):
        edgeT, slotp, idx16 = _pack_core(c, S, edge, idx_j)
        m = dict(common)
        m["edgeT"] = edgeT
        m["slotp"] = slotp
        m["idx16"] = idx16
        in_maps.append(m)

    if TRACE:
        _ensure_ntff_hook()
    res = run_bass_kernel_spmd(
        nc, in_maps, list(range(N_CORES)), trace=TRACE
    )
    LAST_RESULT = res
    out = np.concatenate(
        [res.results[c]["out"][:NPC] for c in range(N_CORES)], axis=0
    )
    return out.astype(np.float32)


# revision 36
# speedup vs baseline: 2.1911x; 1.0698x over previous
"""Trainium2 Bass kernel for GNN message passing.

Computes, for full inputs:
    edge_h = lrelu(lrelu(edge @ We1 + be1) @ We2 + be2)        # [E, 128]
    out    = segment_sum((node @ Wn)[idx_j] * edge_h, seg_i)   # [N, 128]

Strategy (8 NeuronCores, SPMD single program):
  - Shard edges by DESTINATION range (N/8 = 6250 nodes per core). Each core
    produces its own output slice -> no collectives; host concatenates.
  - Phase 1 (replicated on every core): msg = node @ Wn written to two
    internal DRAM tables (halves, so gather indices fit int16), row-major
    fp16 (256B rows -> full-rate gather). msgA slabs are written first so
    pass-A gathers can start while msgB is still being computed.
  - Phase 2, two passes: pass A processes every window's msgA-sourced edges
    (gathers read only msgA), pass B the msgB-sourced ones. Per 128-edge
    tile: edge MLP via PE matmuls, LeakyReLU (Prelu) on the scalar engine,
    one-hot(slot) on DVE, scatter-add = one-hot matmul accumulated in a
    PSUM window [128 slots x 128 feat]. Pass-A windows flush to SBUF
    accumulators; pass-B flush adds the accumulator and stores to DRAM.
  - Gathers (InstDMAGatherAnt) are issued round-robin on 4 SWDGE queues
    (independent Q7 core pairs) with 6 G buffers, so up to 4 descriptor
    generations overlap: ~3-4 ns/idx instead of ~9 ns/idx single-queue.

The tile->window/table schedule is data-dependent and baked into the
program at build time (all cores share it; per-core data is padded to the
common schedule). Programs are cached per schedule within the process.
"""

import os
import sys
import types

import numpy as np

import concourse.bass as bass
import concourse.tile as tile
from concourse.tile import add_dep_helper
from concourse import bacc, mybir
from concourse.bass_utils import run_bass_kernel_spmd

# ---- problem constants (hardcoded per spec) ----
N_NODES = 50000
D_NODE = 128
D_EDGE = 32
D_HID = 128
N_CORES = 8
NPC = N_NODES // N_CORES          # nodes per core = 6250
P = 128                           # partitions
W_PER_CORE = (NPC + P - 1) // P   # 49 windows per core
NEG_SLOPE = 0.01
PAD_SLOT = 300.0                  # slot value that never matches iota 0..127

# node rows padded to a multiple of 1024 so the two msg halves split evenly
N_PAD = ((N_NODES + 1023) // 1024) * 1024    # 50176
N_SLABS = N_PAD // 512                       # 98
HALF = N_PAD // 2                            # 25088 rows per msg table
HALF_SLABS = N_SLABS // 2                    # 49

RUN_MAX = 8          # max tiles per dma_gather (1024 idxs; ring cap, >1920 wedges)
N_QUEUES = 4         # SWDGE queues (Q7 core pairs) for parallel desc-gen
TPB = HALF // P      # 196: msg table stored partition-major, k = p*TPB + t

F32 = mybir.dt.float32
F16 = mybir.dt.float16
I16 = mybir.dt.int16
# data dtype for matmul operands / msg tables (fp16 default; K_DT=f32 to revert)
DT = F32 if os.environ.get("K_DT") == "f32" else F16
DT_NP = np.float32 if os.environ.get("K_DT") == "f32" else np.float16

TRACE = False
LAST_RESULT = None

_PROGRAM_CACHE = {}


def _ensure_ntff_hook():
    """Provide antenv.axon_hooks if this image's antenv lacks it, and
    register the ctypes NTFF profiling hook so trace=True works."""
    try:
        from antenv.axon_hooks import get_axon_ntff_profile_hook  # noqa: F401
        return
    except ImportError:
        pass
    mod = types.ModuleType("antenv.axon_hooks")
    _hook = [None]
    mod.set_axon_ntff_profile_hook = lambda h: _hook.__setitem__(0, h)
    mod.get_axon_ntff_profile_hook = lambda: _hook[0]
    sys.modules["antenv.axon_hooks"] = mod
    import antenv

    antenv.axon_hooks = mod
    try:
        from trn_agent_boot.trn_boot import _ntff_profile_via_ctypes

        mod.set_axon_ntff_profile_hook(
            _ntff_profile_via_ctypes("/opt/axon/libaxon_pjrt.so")
        )
    except Exception:
        pass


# --------------------------------------------------------------------------
# host-side schedule + packing
# --------------------------------------------------------------------------

def cdiv(a, b):
    return (a + b - 1) // b


class Schedule:
    """Common (all-core) static schedule baked into the program.

    Tile sequence = pass A (all windows, table 0) then pass B (table 1).
    """

    def __init__(self, seg_i, idx_j):
        core = seg_i // NPC
        local = seg_i - core * NPC
        win = local // P
        half = (idx_j >= HALF).astype(np.int64)

        cnt = np.zeros((2, N_CORES, W_PER_CORE), dtype=np.int64)
        for c in range(N_CORES):
            m = core == c
            for h in (0, 1):
                cnt[h, c] = np.bincount(
                    win[m & (half == h)], minlength=W_PER_CORE
                )
        a_tiles = np.maximum(cdiv(cnt[0].max(axis=0), P), 1)
        b_tiles = np.maximum(cdiv(cnt[1].max(axis=0), P), 1)

        # tile sequence: pass A (tab 0) then pass B (tab 1)
        win_of, table_of = [], []
        self.block_start = np.zeros((W_PER_CORE, 2), dtype=np.int64)
        self.block_tiles = np.zeros((W_PER_CORE, 2), dtype=np.int64)
        for tab in (0, 1):
            nts = a_tiles if tab == 0 else b_tiles
            for w in range(W_PER_CORE):
                self.block_start[w, tab] = len(win_of)
                self.block_tiles[w, tab] = nts[w]
                for _ in range(int(nts[w])):
                    win_of.append(w)
                    table_of.append(tab)
        win_of = np.array(win_of, dtype=np.int64)
        table_of = np.array(table_of, dtype=np.int64)
        T = len(win_of)
        # first/last per (w, tab) block
        first_of = np.zeros(T, dtype=bool)
        last_of = np.zeros(T, dtype=bool)
        for w in range(W_PER_CORE):
            for tab in (0, 1):
                s = int(self.block_start[w, tab])
                n = int(self.block_tiles[w, tab])
                first_of[s] = True
                last_of[s + n - 1] = True

        self.T = T
        self.win_of = win_of
        self.table_of = table_of
        self.first_of = first_of
        self.last_of = last_of
        self.core, self.local, self.win, self.half = core, local, win, half
        self.n_a_tiles = int(a_tiles.sum())

        # gather runs: same-table spans capped at RUN_MAX (tables are the
        # two contiguous passes, so runs only break at the pass boundary)
        runs = []
        t = 0
        while t < T:
            tab = self.table_of[t]
            e = t
            while e < T and self.table_of[e] == tab and e - t < RUN_MAX:
                e += 1
            runs.append((int(tab), t, e - t))
            t = e
        self.runs = runs

        # chunks: <=4-tile pieces within runs
        chunks = []
        for ri, (tab, t0, L) in enumerate(runs):
            t = t0
            while t < t0 + L:
                nt = min(4, t0 + L - t)
                chunks.append((ri, t, nt))
                t += nt
        self.chunks = chunks

    def key(self):
        return (
            tuple(self.win_of.tolist()),
            tuple(self.table_of.tolist()),
        )


def _pack_core(c, S, edge, idx_j):
    """Per-core padded arrays following the common schedule."""
    T = S.T
    perm = np.full(T * P, -1, dtype=np.int64)
    for w in range(W_PER_CORE):
        for tab in (0, 1):
            sel = np.flatnonzero(
                (S.core == c) & (S.win == w) & (S.half == tab)
            )
            n = len(sel)
            s0 = S.block_start[w, tab] * P
            cap = S.block_tiles[w, tab] * P
            assert n <= cap, f"schedule overflow c={c} w={w} tab={tab}"
            perm[s0:s0 + n] = sel

    valid = perm >= 0
    pidx = np.where(valid, perm, 0)

    # idx16 [128, T*8]: per tile-order flattening i=t*128+p -> [i%16, i//16]
    loc = (idx_j[pidx] - S.table_of[np.arange(T * P) // P] * HALF).astype(
        np.int64
    )
    loc[~valid] = 0
    assert (loc >= 0).all() and (loc < HALF).all()
    if os.environ.get("K_REMAP") != "0":
        # msg tables partition-major: node j at position (j%128)*TPB + j//128
        loc = (loc % P) * TPB + loc // P
    idx16 = loc.astype(np.int16).reshape(T * 8, 16).T  # [16, T*8]
    idx16 = np.tile(idx16, (8, 1)).copy()              # [128, T*8]

    # chunk-packed edge features and slots
    n_ch = len(S.chunks)
    edgeT = np.zeros((n_ch, D_EDGE, 512), dtype=DT_NP)
    slotp = np.full((n_ch, P, 4), PAD_SLOT, dtype=DT_NP)
    slots_all = (S.local[pidx] % P).astype(DT_NP)
    slots_all[~valid] = PAD_SLOT
    ev = np.zeros((T * P, D_EDGE), dtype=DT_NP)
    ev[valid] = edge[pidx[valid]]
    for k, (ri, t0, nt) in enumerate(S.chunks):
        edgeT[k, :, :nt * P] = ev[t0 * P:(t0 + nt) * P].T
        for t in range(nt):
            slotp[k, :, t] = slots_all[(t0 + t) * P:(t0 + t + 1) * P]
    return edgeT, slotp, idx16


# --------------------------------------------------------------------------
# device program
# --------------------------------------------------------------------------

def _build_program(S):
    T = S.T
    n_ch = len(S.chunks)
    dbg_skip_p1 = os.environ.get("K_SKIP_P1") == "1"
    dbg_skip_gather = os.environ.get("K_SKIP_GATHER") == "1"
    dbg_max_chunks = int(os.environ.get("K_MAX_CHUNKS", "999999"))
    dbg_stages = int(os.environ.get("K_STAGES", "99"))

    nc = bacc.Bacc(
        "TRN2", target_bir_lowering=False, debug=False, num_devices=N_CORES,
        num_swdge_queues=N_QUEUES,
    )

    # ---- I/O ----
    nodeT_h = nc.dram_tensor("nodeT", [P, N_PAD], DT, kind="ExternalInput").ap()
    Wn_h = nc.dram_tensor("Wn", [D_NODE, D_HID], DT, kind="ExternalInput").ap()
    We1_h = nc.dram_tensor("We1p", [D_EDGE, D_HID], DT, kind="ExternalInput").ap()
    We2_h = nc.dram_tensor("We2", [D_HID, D_HID], DT, kind="ExternalInput").ap()
    be1_h = nc.dram_tensor("be1c", [P, 1], F32, kind="ExternalInput").ap()
    be2_h = nc.dram_tensor("be2bc", [P, 512], F32, kind="ExternalInput").ap()
    iota_h = nc.dram_tensor("iota", [P, P], DT, kind="ExternalInput").ap()
    ones_h = nc.dram_tensor("ones1", [1, P], DT, kind="ExternalInput").ap()
    be2r_h = nc.dram_tensor("be2r", [1, 512], DT, kind="ExternalInput").ap()
    edgeT_h = nc.dram_tensor(
        "edgeT", [n_ch, D_EDGE, 512], DT, kind="ExternalInput"
    ).ap()
    slot_h = nc.dram_tensor(
        "slotp", [n_ch, P, 4], DT, kind="ExternalInput"
    ).ap()
    idx16_h = nc.dram_tensor(
        "idx16", [P, T * 8], I16, kind="ExternalInput"
    ).ap()
    out_h = nc.dram_tensor(
        "out", [W_PER_CORE * P, D_HID], F32, kind="ExternalOutput"
    ).ap()

    msg_h = [
        nc.dram_tensor("msgA", [HALF, D_HID], DT).ap(),
        nc.dram_tensor("msgB", [HALF, D_HID], DT).ap(),
    ]

    LR = mybir.ActivationFunctionType.Prelu

    with tile.TileContext(nc) as tc:
        with tc.tile_pool(name="consts", bufs=1) as cpool:
            Wn_sb = cpool.tile([D_NODE, D_HID], DT)
            nc.sync.dma_start(Wn_sb[:], Wn_h[:])
            We1_sb = cpool.tile([D_EDGE, D_HID], DT)
            nc.sync.dma_start(We1_sb[:], We1_h[:])
            We2_sb = cpool.tile([D_HID, D_HID], DT)
            nc.sync.dma_start(We2_sb[:], We2_h[:])
            be1_sb = cpool.tile([P, 1], F32)
            nc.sync.dma_start(be1_sb[:], be1_h[:])
            be2_sb = cpool.tile([P, 512], F32)
            nc.sync.dma_start(be2_sb[:], be2_h[:])
            iota_sb = cpool.tile([P, P], DT)
            nc.sync.dma_start(iota_sb[:], iota_h[:])
            ones_sb = cpool.tile([1, P], DT)
            nc.sync.dma_start(ones_sb[:], ones_h[:])
            be2r_sb = cpool.tile([1, 512], DT)
            nc.sync.dma_start(be2r_sb[:], be2r_h[:])
            # whole idx table up front (frees the sync queue during phase 2
            # and removes per-run ix waits on the gather engine)
            ix_all = cpool.tile([P, T * 8], I16)
            nc.sync.dma_start(ix_all[:], idx16_h[:])

            # ---- phase 1: msg = node @ Wn (1024-col slabs; A half first) ----
            # slab list: per table, 24 x 1024 + 1 x 512 (HALF = 25088 rows)
            slabs = []
            for tab in (0, 1):
                for k in range(24):
                    slabs.append((tab, k * 1024, 1024))
                slabs.append((tab, 24 * 1024, 512))
            p1_stores = [[], []]   # per table
            with (
                tc.tile_pool(name="p1_in", bufs=6) as p1in,
                tc.tile_pool(name="p1_stage", bufs=6) as p1st,
                tc.tile_pool(name="p1_psum", bufs=4, space="PSUM") as p1ps,
            ):
                for g, (tab, r0, ncols) in enumerate(
                    [] if dbg_skip_p1 else slabs
                ):
                    col0 = tab * HALF + r0
                    nt_sb = p1in.tile([P, 1024], DT, tag="nodeT")
                    # phase-1 loads on the gpsimd queue (idle in phase 1),
                    # stores on scalar: the sync queue stays free for phase 2
                    nc.gpsimd.dma_start(
                        nt_sb[:, :ncols], nodeT_h[:, col0:col0 + ncols]
                    )
                    ps = p1ps.tile([P, 1024], F32, tag="p1ps")
                    for t in range(ncols // P):
                        nc.tensor.matmul(
                            ps[:, t * P:(t + 1) * P],
                            lhsT=nt_sb[:, t * P:(t + 1) * P],
                            rhs=Wn_sb[:],
                            start=True,
                            stop=True,
                        )
                    stage = p1st.tile([P, 1024], DT, tag="p1stage")
                    if g % 2 == 0:
                        nc.vector.tensor_copy(stage[:, :ncols], ps[:, :ncols])
                    else:
                        nc.scalar.activation(
                            stage[:, :ncols], ps[:, :ncols],
                            mybir.ActivationFunctionType.Copy,
                        )
                    # table is partition-major: slab rows j=r0+t*128+p land at
                    # k = p*TPB + (r0//128 + t) -> contiguous 2KB per partition
                    if os.environ.get("K_REMAP") != "0":
                        t0 = r0 // P
                        dst = msg_h[tab][:].rearrange(
                            "(p t) f -> p t f", t=TPB
                        )[:, t0:t0 + ncols // P, :]
                    else:
                        dst = msg_h[tab][r0:r0 + ncols, :].rearrange(
                            "(t p) f -> p t f", p=P
                        )
                    srcap = stage[:, :ncols].rearrange(
                        "p (t f) -> p t f", t=ncols // P
                    )
                    st_inst = nc.scalar.dma_start(dst, srcap)
                    p1_stores[tab].append(st_inst.ins)

            # ---- phase 2: pass A then pass B ----
            with (
                tc.tile_pool(name="p2_in", bufs=8) as p2in,
                tc.tile_pool(name="p2_g", bufs=16) as p2g,
                tc.tile_pool(name="p2_mid", bufs=3) as p2mid,
                tc.tile_pool(name="p2_acc", bufs=1) as accp,
                tc.tile_pool(name="h1_psum", bufs=2, space="PSUM") as h1ps,
                tc.tile_pool(name="h2_psum", bufs=2, space="PSUM") as h2ps,
                tc.tile_pool(name="out_psum", bufs=2, space="PSUM") as outps,
                tc.tile_pool(name="out_stage", bufs=3) as outst,
            ):
                cur_out = {}
                acc = {}
                reg_full = nc.gpsimd.to_reg(RUN_MAX * P)

                chunks_by_run = {}
                for k, (ri, t0, nt) in enumerate(S.chunks):
                    chunks_by_run.setdefault(ri, []).append((k, t0, nt))

                first_run_of_tab = {}
                for ri, (tab, rt0, L) in enumerate(S.runs):
                    if tab not in first_run_of_tab:
                        first_run_of_tab[tab] = ri

                for ri, (tab, rt0, L) in enumerate(S.runs):
                    G = p2g.tile(
                        [P, RUN_MAX * P], DT, tag="G", name=f"G_r{ri}"
                    )
                    if dbg_skip_gather:
                        nc.gpsimd.memset(G[:, :L * P], 0.5)
                    else:
                        g_inst = nc.gpsimd.dma_gather(
                            G[:, :L * P].rearrange("p (g f) -> p g f", f=P),
                            msg_h[tab][:],
                            ix_all[:, rt0 * 8:(rt0 + L) * 8],
                            num_idxs=L * P,
                            num_idxs_reg=(
                                reg_full if L == RUN_MAX else L * P
                            ),
                            elem_size=P,
                            elem_step=P,
                            queue_num=ri % N_QUEUES,
                        )
                        if ri == first_run_of_tab.get(tab):
                            for st in p1_stores[tab]:
                                add_dep_helper(
                                    g_inst.ins, st, sync=True,
                                    reason=f"gather after msg{tab} stores",
                                )

                    # one edgeT/slot DMA covers the whole run's chunks
                    rchunks = [c for c in chunks_by_run[ri]
                               if c[0] < dbg_max_chunks]
                    if rchunks:
                        k0 = rchunks[0][0]
                        nk = len(rchunks)
                        et_run = p2in.tile(
                            [D_EDGE, 2 * 512], DT, tag="edgeT"
                        )
                        nc.sync.dma_start(
                            et_run[:, :nk * 512].rearrange(
                                "e (k c) -> e k c", k=nk
                            ),
                            edgeT_h[k0:k0 + nk, :, :].rearrange(
                                "k e c -> e k c"
                            ),
                        )
                        sl_run = p2in.tile([P, 2 * 4], DT, tag="slot")
                        nc.sync.dma_start(
                            sl_run[:, :nk * 4].rearrange(
                                "p (k c) -> p k c", k=nk
                            ),
                            slot_h[k0:k0 + nk, :, :].rearrange(
                                "k p c -> p k c"
                            ),
                        )

                    for (k, t0, nt) in rchunks:
                        ncols = nt * P
                        goff = (t0 - rt0) * P
                        kk = k - rchunks[0][0]
                        et_sb = et_run[:, kk * 512:kk * 512 + 512]
                        sl_sb = sl_run[:, kk * 4:kk * 4 + 4]

                        # h1 = lrelu(edge @ We1 + be1), feature-major [h x e]
                        ps1 = h1ps.tile([P, 512], F32, tag="h1ps")
                        nc.tensor.matmul(
                            ps1[:, :ncols],
                            lhsT=We1_sb[:],
                            rhs=et_sb[:, :ncols],
                            start=True,
                            stop=True,
                        )
                        h1f = p2mid.tile([P, 512], DT, tag="h1f")
                        if dbg_stages >= 2:
                            nc.scalar.activation(
                                h1f[:, :ncols], ps1[:, :ncols], LR,
                                bias=be1_sb[:], scale=1.0, alpha=NEG_SLOPE,
                            )
                        else:
                            nc.vector.tensor_copy(h1f[:, :ncols], ps1[:, :ncols])
                        if dbg_stages < 3:
                            continue

                        # h2 = h1.T @ We2 + be2, edge-major [e x h];
                        # the bias lands via a K=1 accumulate matmul (PE, not DVE)
                        ps2 = h2ps.tile([P, 512], F32, tag="h2ps")
                        use_bias_mm = os.environ.get("K_BIAS_MM") == "1"
                        mm2s = []
                        for t in range(nt):
                            mm2s.append(nc.tensor.matmul(
                                ps2[:, t * P:(t + 1) * P],
                                lhsT=h1f[:, t * P:(t + 1) * P],
                                rhs=We2_sb[:],
                                start=True,
                                stop=not use_bias_mm,
                            ))
                        if use_bias_mm:
                            bmm = nc.tensor.matmul(
                                ps2[:, :ncols],
                                lhsT=ones_sb[:],
                                rhs=be2r_sb[:, :ncols],
                                start=False,
                                stop=True,
                            )
                            for m in mm2s:
                                add_dep_helper(
                                    bmm.ins, m.ins, sync=False,
                                    reason="bias accumulate after mm2 blocks",
                                )
                        else:
                            nc.vector.tensor_tensor(
                                ps2[:, :ncols], in0=ps2[:, :ncols],
                                in1=be2_sb[:, :ncols], op=mybir.AluOpType.add,
                            )
                        eh = p2mid.tile([P, 512], DT, tag="eh")
                        if dbg_stages >= 4:
                            nc.scalar.activation(
                                eh[:, :ncols], ps2[:, :ncols], LR,
                                scale=1.0, alpha=NEG_SLOPE,
                            )
                        else:
                            nc.vector.tensor_copy(eh[:, :ncols], ps2[:, :ncols])
                        if dbg_stages < 5:
                            continue

                        # onehot[e, s] = (slot[e] == s), all nt tiles in one op
                        oh = p2mid.tile([P, 512], DT, tag="oh")
                        if dbg_stages >= 5:
                            nc.vector.tensor_tensor(
                                oh[:, :ncols].rearrange(
                                    "p (t f) -> p t f", t=nt
                                ),
                                in0=sl_sb[:, :nt].rearrange(
                                    "p (t o) -> p t o", o=1
                                ).to_broadcast([P, nt, P]),
                                in1=iota_sb[:].rearrange(
                                    "p (o f) -> p o f", o=1
                                ).to_broadcast([P, nt, P]),
                                op=mybir.AluOpType.is_equal,
                            )
                        else:
                            nc.gpsimd.memset(oh[:, :ncols], 0.0)
                        if dbg_stages < 6:
                            continue

                        # product = gathered msg * edge_h
                        pr = p2mid.tile([P, 512], DT, tag="pr")
                        nc.vector.tensor_tensor(
                            pr[:, :ncols],
                            in0=G[:, goff:goff + ncols],
                            in1=eh[:, :ncols],
                            op=mybir.AluOpType.mult,
                        )

                        if dbg_stages < 7:
                            continue
                        # scatter: out_w[s, f] += onehot[:, t].T @ product[:, t]
                        for t in range(nt):
                            i = t0 + t
                            w = int(S.win_of[i])
                            if S.first_of[i]:
                                cur_out[w] = outps.tile(
                                    [P, P], F32, tag="outp",
                                    name=f"outp_w{w}t{tab}"
                                )
                            nc.tensor.matmul(
                                cur_out[w][:],
                                lhsT=oh[:, t * P:(t + 1) * P],
                                rhs=pr[:, t * P:(t + 1) * P],
                                start=bool(S.first_of[i]),
                                stop=bool(S.last_of[i]),
                            )
                            if S.last_of[i]:
                                if tab == 0:
                                    # pass A: stash partial in SBUF (scalar
                                    # engine copy; DVE is the busier one)
                                    a = accp.tile(
                                        [P, P], F32, tag=f"acc_w{w}",
                                        name=f"acc_w{w}"
                                    )
                                    nc.scalar.activation(
                                        a[:], cur_out[w][:],
                                        mybir.ActivationFunctionType.Copy,
                                    )
                                    acc[w] = a
                                else:
                                    # pass B: add pass-A partial, store out
                                    st = outst.tile(
                                        [P, P], F32, tag="outstage",
                                        name=f"outst_w{w}"
                                    )
                                    nc.vector.tensor_tensor(
                                        st[:], in0=cur_out[w][:],
                                        in1=acc[w][:],
                                        op=mybir.AluOpType.add,
                                    )
                                    nc.scalar.dma_start(
                                        out_h[w * P:(w + 1) * P, :], st[:]
                                    )
                                del cur_out[w]

    nc.compile()
    return nc


# --------------------------------------------------------------------------
# entry point
# --------------------------------------------------------------------------

def kernel(node, edge, Wn, We1, be1, We2, be2, seg_i, idx_j):
    global LAST_RESULT
    node = np.asarray(node, dtype=np.float32)
    edge = np.asarray(edge, dtype=np.float32)
    Wn = np.asarray(Wn, dtype=np.float32)
    We1 = np.asarray(We1, dtype=np.float32)
    be1 = np.asarray(be1, dtype=np.float32)
    We2 = np.asarray(We2, dtype=np.float32)
    be2 = np.asarray(be2, dtype=np.float32)
    seg_i = np.asarray(seg_i, dtype=np.int32)
    idx_j = np.asarray(idx_j, dtype=np.int32)

    S = Schedule(seg_i.astype(np.int64), idx_j.astype(np.int64))
    key = S.key()
    if key not in _PROGRAM_CACHE:
        _PROGRAM_CACHE[key] = _build_program(S)
    nc = _PROGRAM_CACHE[key]

    nodeT = np.zeros((P, N_PAD), dtype=DT_NP)
    nodeT[:, :N_NODES] = node.T
    iota = np.broadcast_to(np.arange(P, dtype=DT_NP), (P, P)).copy()
    common = {
        "nodeT": nodeT,
        "Wn": Wn.astype(DT_NP),
        "We1p": We1.astype(DT_NP),
        "We2": We2.astype(DT_NP),
        "be1c": be1.reshape(P, 1).copy(),
        "be2bc": np.broadcast_to(
            np.tile(be2, 4), (P, 512)
        ).astype(np.float32).copy(),
        "iota": iota,
        "ones1": np.ones((1, P), dtype=DT_NP),
        "be2r": np.tile(be2, 4).reshape(1, 512).astype(DT_NP),
    }
    in_maps = []
    for c in range(N_CORES):
        edgeT, slotp, idx16 = _pack_core(c, S, edge, idx_j)
        m = dict(common)
        m["edgeT"] = edgeT
        m["slotp"] = slotp
        m["idx16"] = idx16
        in_maps.append(m)

    if TRACE:
        _ensure_ntff_hook()
    res = run_bass_kernel_spmd(
        nc, in_maps, list(range(N_CORES)), trace=TRACE
    )
    LAST_RESULT = res
    out = np.concatenate(
        [res.results[c]["out"][:NPC] for c in range(N_CORES)], axis=0
    )
    return out.astype(np.float32)
